# revision 1
# baseline (speedup 1.0000x reference)
"""Causal self-attention (RMSNorm QK, key-gated ALiBi bias) on 8 TRN2 cores.

Sharding: data-parallel over batch (2) x tensor-parallel over heads (4 groups
of 4 heads) = 8 cores. Each core computes a partial c_proj output for its
batch; the host sums the 4 head-group partials per batch.

Device kernel (identical SPMD program, per-core data):
  P2  QKV projections (f32r matmuls, x^T resident in SBUF)
      q^T/k^T per 2-head pack -> per-head augmented tiles
      [66, T]: rows 0-63 rms-normalized q (or k), rows 64-65 bias terms.
  P3  key-gate: glog = u_n . k_norm / sqrt(D); gate = softplus via exp/ln;
      bias rows a[j] = j*w[j], -w[j] with w = softplus(omega)*slope*gate.
  P4  scores^T[j, i] = k_aug . q_aug (bias folded into the contraction),
      causal stair mask added pre-exp on diagonal blocks, exp on ACT,
      PV matmul with a ones-column in v giving softmax denominators free.
  P5  y normalized via exp(-ln(denom)) broadcast, packed 2 heads/tile,
      c_proj matmul, partial output to DRAM.
"""

import sys

if "/opt/trn_rl_repo" not in sys.path:
    sys.path.insert(0, "/opt/trn_rl_repo")

import math

import numpy as np

B, T, C = 2, 2048, 1024
H, D = 16, 64
HLOC = 4           # heads per core
HD = HLOC * D      # 256
NCH = 512          # i-chunk width
NT = T // NCH      # 4 i-chunks
JT = T // 128      # 16 j-tiles
KC = C // 128      # 8 contraction chunks
EPS_RMS = 1e-5
U_L2_EPS = 1e-6
NEG_BIG = -1.0e30

_cache = {}


def _get_alibi_slopes(n_heads):
    def pow2(n):
        start = 2 ** (-(2 ** (-(math.log2(n) - 3))))
        return [start * start**i for i in range(n)]

    if math.log2(n_heads).is_integer():
        return pow2(n_heads)
    c = 2 ** math.floor(math.log2(n_heads))
    s = pow2(c)
    extra = _get_alibi_slopes(2 * c)
    return s + extra[0::2][: n_heads - c]


def _build_program():
    import concourse.bass as bass
    import concourse.mybir as mybir
    import concourse.tile as tile
    from concourse.alu_op_type import AluOpType
    from concourse.vector_clock import ScopedClock

    F32 = mybir.dt.float32
    F32R = mybir.dt.float32r
    BF16 = mybir.dt.bfloat16
    AF = mybir.ActivationFunctionType
    MUL = AluOpType.mult
    ADD = AluOpType.add
    SUB = AluOpType.subtract

    class PatchedTileContext(tile.TileContext):
        """Tail drain split into nops carrying <=2 sem waits each (this
        walrus build rejects CTRL instructions with more)."""

        def _drain_and_barrier(self, tick_clock, wait_clock):
            nc = self.nc
            probe = nc.sync.nop(nofuse=True)
            wait_clock.add_sem_waits(
                probe.ins, ScopedClock({None: tick_clock.global_clock})
            )
            si = probe.ins.sync_info
            waits = list(si.on_wait or []) if si is not None else []
            if len(waits) > 2:
                si.on_wait = waits[:2]
                rest = waits[2:]
                for i in range(0, len(rest), 2):
                    extra = nc.sync.nop(nofuse=True)
                    esi = extra.ins.sync_info
                    chunk = rest[i : i + 2]
                    if esi is None:
                        extra.ins.sync_info = mybir.SyncInfo(
                            on_wait=chunk, on_update=[]
                        )
                    else:
                        esi.on_wait = (esi.on_wait or []) + chunk
            nc.sync.drain()
            nc.all_engine_barrier()
            assert self.sems is not None
            popped = nc._tile_sem_poison_stack.pop()
            assert popped is self._sem_poison
            nc.clear_and_free_semaphores(list(self.sems.allocated().values()))
            nc.all_engine_barrier()

    def split_excess_waits(nc, max_waits=1):
        for f in nc.m.functions:
            for blk in f.blocks:
                new_insts = []
                for inst in blk.instructions:
                    si = inst.sync_info
                    if si is not None and si.on_wait and len(si.on_wait) > max_waits:
                        waits = list(si.on_wait)
                        si.on_wait = waits[-max_waits:]
                        rest = waits[:-max_waits]
                        for i in range(0, len(rest), max_waits):
                            nop = mybir.InstNoOp(
                                name=f"I-waitsplit-{nc.next_id()}",
                                ins=[],
                                outs=[],
                                engine=inst.engine,
                                sync_info=mybir.SyncInfo(
                                    on_wait=rest[i : i + max_waits], on_update=[]
                                ),
                            )
                            nc.register_instruction(nop)
                            new_insts.append(nop)
                    new_insts.append(inst)
                blk.instructions = new_insts

    nc = bass.Bass(trn_type="TRN2", num_devices=8, debug=False)

    # ---- DRAM I/O (per-core shards supplied by the host) ----
    d_xT = nc.dram_tensor("xT", [C, T], F32, kind="ExternalInput")
    d_wq = nc.dram_tensor("wq", [C, HD], F32, kind="ExternalInput")
    d_wk = nc.dram_tensor("wk", [C, HD], F32, kind="ExternalInput")
    d_wv = nc.dram_tensor("wv", [C, HD], F32, kind="ExternalInput")
    d_wproj = nc.dram_tensor("wproj", [HD, C], F32, kind="ExternalInput")
    d_ucol = nc.dram_tensor("ucol", [D, HLOC], F32, kind="ExternalInput")
    d_omg = nc.dram_tensor("omg", [HLOC, 1], F32, kind="ExternalInput")
    d_negomg = nc.dram_tensor("negomg", [HLOC, 1], F32, kind="ExternalInput")
    d_iota4 = nc.dram_tensor("iota4", [HLOC, T], F32, kind="ExternalInput")
    d_oh16 = nc.dram_tensor("oh16", [1, 16], F32, kind="ExternalInput")
    d_iota = nc.dram_tensor("iota", [1, T], F32, kind="ExternalInput")
    d_ones = nc.dram_tensor("ones_row", [1, T], F32, kind="ExternalInput")
    d_ones4 = nc.dram_tensor("ones4", [128, HLOC], BF16, kind="ExternalInput")
    d_stair = nc.dram_tensor("stair", [128, 128], F32, kind="ExternalInput")
    d_ssqw = nc.dram_tensor("ssqw", [128, 2], F32, kind="ExternalInput")
    d_qw8 = nc.dram_tensor("qw8", [128, 1], F32, kind="ExternalInput")
    d_kw = nc.dram_tensor("kw", [128, 1], F32, kind="ExternalInput")
    d_out = nc.dram_tensor("out", [T, C], F32, kind="ExternalOutput")

    with PatchedTileContext(nc) as tc:
        from contextlib import ExitStack

        with ExitStack() as top:
            persist = top.enter_context(tc.tile_pool(name="persist", bufs=1))

            # ---- persistent SBUF tensors ----
            q_aug = [persist.tile([68, T], F32R, tag=f"qaug{h}", name=f"qaug{h}") for h in range(HLOC)]
            k_aug = [persist.tile([68, T], F32R, tag=f"kaug{h}", name=f"kaug{h}") for h in range(HLOC)]
            v_sb = [
                persist.tile([128, HLOC * 65], BF16, tag=f"vsb{t}", name=f"vsb{t}") for t in range(JT)
            ]
            stair = persist.tile([128, 128], F32, tag="stair", name="stair")
            nc.sync.dma_start(stair[:], d_stair[:])
            ssqw = persist.tile([128, 2], F32R, tag="ssqw", name="ssqw")
            nc.sync.dma_start(ssqw[:], d_ssqw[:].bitcast(F32R))
            ucol = persist.tile([D, HLOC], F32R, tag="ucol", name="ucol")
            nc.sync.dma_start(ucol[:], d_ucol[:].bitcast(F32R))
            omg = persist.tile([HLOC, 1], F32, tag="omg", name="omg")
            nc.sync.dma_start(omg[:], d_omg[:])
            negomg = persist.tile([HLOC, 1], F32, tag="negomg", name="negomg")
            oh16 = persist.tile([1, 16], F32R, tag="oh16", name="oh16")
            nc.sync.dma_start(oh16[:], d_oh16[:].bitcast(F32R))
            nc.sync.dma_start(negomg[:], d_negomg[:])
            qw8 = persist.tile([128, 1], F32, tag="qw8", name="qw8")
            nc.sync.dma_start(qw8[:], d_qw8[:])
            kw = persist.tile([128, 1], F32, tag="kw", name="kw")
            nc.sync.dma_start(kw[:], d_kw[:])
            eps_col = persist.tile([128, 1], F32, tag="eps", name="eps")
            nc.vector.memset(eps_col[:], EPS_RMS)
            neghalf_col = persist.tile([128, 1], F32, tag="neghalf", name="neghalf")
            nc.vector.memset(neghalf_col[:], -0.5)
            neg1_col = persist.tile([128, 1], F32, tag="neg1", name="neg1")
            nc.vector.memset(neg1_col[:], -1.0)
            one_col = persist.tile([128, 1], F32, tag="onec", name="onec")
            nc.vector.memset(one_col[:], 1.0)

            # aug fixed rows: q rows 64 (ones) / 65 (iota)
            for h in range(HLOC):
                nc.sync.dma_start(q_aug[h][64:65, :], d_ones[:].bitcast(F32R))
                nc.sync.dma_start(q_aug[h][65:66, :], d_ones[:].bitcast(F32R))
                nc.sync.dma_start(q_aug[h][66:67, :], d_iota[:].bitcast(F32R))
                nc.sync.dma_start(q_aug[h][67:68, :], d_iota[:].bitcast(F32R))

            # v ones columns
            for t in range(JT):
                dst = v_sb[t][:].rearrange("p (h d) -> p h d", h=HLOC)[:, :, 64:65]
                nc.sync.dma_start(dst, d_ones4[:].rearrange("p (h o) -> p h o", o=1))

            # ================= P2: QKV projections =================
            with ExitStack() as p2:
                xpool = p2.enter_context(tc.tile_pool(name="xT", bufs=1))
                xT = []
                for cc in range(KC):
                    t = xpool.tile([128, T], F32R, tag=f"xT{cc}", name=f"xT{cc}")
                    nc.sync.dma_start(
                        t[:], d_xT[128 * cc : 128 * cc + 128, :].bitcast(F32R)
                    )
                    xT.append(t)

                wpool = p2.enter_context(tc.tile_pool(name="w", bufs=1))
                qk_ps = p2.enter_context(
                    tc.tile_pool(name="qkps", bufs=2, space="PSUM")
                )
                ssq_ps = p2.enter_context(
                    tc.tile_pool(name="ssqps", bufs=2, space="PSUM")
                )
                sq_pool = p2.enter_context(tc.tile_pool(name="qsq", bufs=2))
                rsq_pool = p2.enter_context(tc.tile_pool(name="rsq", bufs=3))
                rep_pool = p2.enter_context(tc.tile_pool(name="rep", bufs=4))
                dram = p2.enter_context(
                    tc.tile_pool(name="dram", bufs=16, space="DRAM")
                )

                w_sb = {}
                for name, dten in (("q", d_wq), ("k", d_wk)):
                    for cc in range(KC):
                        wt = wpool.tile([128, HD], F32R, tag=f"w{name}{cc}", name=f"w{name}{cc}")
                        nc.sync.dma_start(
                            wt[:], dten[128 * cc : 128 * cc + 128, :].bitcast(F32R)
                        )
                        w_sb[(name, cc)] = wt

                for p in range(2):  # head pack
                    for n in range(NT):  # T chunk
                        sl = slice(NCH * n, NCH * n + NCH)
                        for name, wcol, proj_w in (
                            ("q", qw8, "q"),
                            ("k", kw, "k"),
                        ):
                            ps = qk_ps.tile([128, NCH], F32, tag="qk", name="qk")
                            for cc in range(KC):
                                nc.tensor.matmul(
                                    ps[:],
                                    w_sb[(name, cc)][:, 128 * p : 128 * p + 128],
                                    xT[cc][:, sl],
                                    start=(cc == 0),
                                    stop=(cc == KC - 1),
                                )
                            # sum of squares per head slot -> mean
                            qsq = sq_pool.tile([128, NCH], F32R, tag="qsq", name="qsq")
                            nc.scalar.activation(qsq[:], ps[:], AF.Square)
                            ssq = ssq_ps.tile([2, NCH], F32, tag="ssq", name="ssq")
                            nc.tensor.matmul(
                                ssq[:], ssqw[:], qsq[:], start=True, stop=True
                            )
                            # rsqrt(mean + eps) = exp(-0.5 ln(mean + eps))
                            rsq = rsq_pool.tile([2, NCH], F32, tag="rsq", name="rsq")
                            nc.scalar.activation(
                                rsq[:], ssq[:], AF.Ln, bias=eps_col[0:2, :]
                            )
                            nc.scalar.activation(
                                rsq[:], rsq[:], AF.Exp, scale=neghalf_col[0:2, :]
                            )
                            for s in range(2):  # head slot in pack
                                h = 2 * p + s
                                drow = dram.tile([1, NCH], F32, tag="drsq", name="drsq")
                                nc.sync.dma_start(drow[:], rsq[s : s + 1, :])
                                rep = rep_pool.tile([64, NCH], F32, tag="rep", name="rep")
                                nc.sync.dma_start(
                                    rep[:], drow[0:1, :].partition_broadcast(64)
                                )
                                aug = q_aug[h] if name == "q" else k_aug[h]
                                nc.vector.scalar_tensor_tensor(
                                    aug[0:64, sl],
                                    ps[64 * s : 64 * s + 64, :],
                                    wcol[0:64, :],
                                    rep[:],
                                    MUL,
                                    MUL,
                                )

                # free q/k weights, load v weights
                wv_sb = []
                for cc in range(KC):
                    wt = wpool.tile([128, HD], F32R, tag=f"wq{cc}", name=f"wv{cc}")
                    nc.sync.dma_start(
                        wt[:], d_wv[128 * cc : 128 * cc + 128, :].bitcast(F32R)
                    )
                    wv_sb.append(wt)
                v_ps_pool = p2.enter_context(
                    tc.tile_pool(name="vps", bufs=2, space="PSUM")
                )
                for t in range(JT):
                    vps = v_ps_pool.tile([128, HD], F32, tag="vps", name="vps")
                    for cc in range(KC):
                        nc.tensor.matmul(
                            vps[:],
                            xT[cc][:, 128 * t : 128 * t + 128],
                            wv_sb[cc][:],
                            start=(cc == 0),
                            stop=(cc == KC - 1),
                        )
                    dst = v_sb[t][:].rearrange("p (h d) -> p h d", h=HLOC)[:, :, 0:64]
                    nc.vector.tensor_copy(
                        dst, vps[:].rearrange("p (h d) -> p h d", h=HLOC)
                    )

                # ================= P3: key gate / bias rows =================
                glog_ps = p2.enter_context(
                    tc.tile_pool(name="glps", bufs=1, space="PSUM")
                )
                gate4_ps = p2.enter_context(
                    tc.tile_pool(name="g4ps", bufs=1, space="PSUM")
                )
                grow_pool = p2.enter_context(tc.tile_pool(name="grow", bufs=2))
                iot_pool = p2.enter_context(tc.tile_pool(name="iot", bufs=2))
                for n in range(NT):
                    sl = slice(NCH * n, NCH * n + NCH)
                    g4 = gate4_ps.tile([HLOC, NCH], F32, tag="g4", name="g4")
                    for h in range(HLOC):
                        gl = glog_ps.tile([1, NCH], F32, tag="glog", name="glog")
                        nc.tensor.matmul(
                            gl[:],
                            ucol[:, h : h + 1],
                            k_aug[h][0:64, sl],
                            start=True,
                            stop=True,
                        )
                        gsc = grow_pool.tile([1, NCH], F32, tag="gsc", name="gsc")
                        nc.scalar.activation(gsc[:], gl[:], AF.Exp)
                        gate = grow_pool.tile([1, NCH], F32R, tag="gate", name="gate")
                        nc.scalar.activation(
                            gate[:], gsc[:], AF.Ln, bias=one_col[0:1, :]
                        )
                        nc.tensor.matmul(
                            g4[:],
                            oh16[0:1, 4 * h : 4 * h + 4],
                            gate[:],
                            start=(h == 0),
                            stop=(h == HLOC - 1),
                            skip_group_check=True,
                        )
                    iots = iot_pool.tile([HLOC, NCH], F32, tag="iot", name="iot")
                    nc.sync.dma_start(iots[:], d_iota4[:, sl])
                    a4f = grow_pool.tile([HLOC, NCH], F32, tag="a4f", name="a4f")
                    nc.vector.scalar_tensor_tensor(
                        a4f[:], g4[:], omg[:], iots[:], MUL, MUL
                    )
                    w4f = grow_pool.tile([HLOC, NCH], F32, tag="w4f", name="w4f")
                    nc.vector.tensor_scalar_mul(w4f[:], g4[:], negomg[:])
                    a_hi = grow_pool.tile([HLOC, NCH], F32R, tag="a_hi", name="a_hi")
                    nc.vector.tensor_copy(a_hi[:], a4f[:])
                    a_lo = grow_pool.tile([HLOC, NCH], F32R, tag="a_lo", name="a_lo")
                    nc.vector.scalar_tensor_tensor(
                        a_lo[:], a4f[:], 1.0, a_hi[:].bitcast(F32), MUL, SUB
                    )
                    w_hi = grow_pool.tile([HLOC, NCH], F32R, tag="w_hi", name="w_hi")
                    nc.vector.tensor_copy(w_hi[:], w4f[:])
                    w_lo = grow_pool.tile([HLOC, NCH], F32R, tag="w_lo", name="w_lo")
                    nc.vector.scalar_tensor_tensor(
                        w_lo[:], w4f[:], 1.0, w_hi[:].bitcast(F32), MUL, SUB
                    )
                    for h in range(HLOC):
                        nc.sync.dma_start(k_aug[h][64:65, sl], a_hi[h : h + 1, :])
                        nc.sync.dma_start(k_aug[h][65:66, sl], a_lo[h : h + 1, :])
                        nc.sync.dma_start(k_aug[h][66:67, sl], w_hi[h : h + 1, :])
                        nc.sync.dma_start(k_aug[h][67:68, sl], w_lo[h : h + 1, :])

            # ================= P4 + P5: attention & projection =============
            with ExitStack() as p4:
                s_ps_pool = p4.enter_context(
                    tc.tile_pool(name="sps", bufs=3, space="PSUM")
                )
                y_ps_pool = p4.enter_context(
                    tc.tile_pool(name="yps", bufs=2, space="PSUM")
                )
                o_ps_pool = p4.enter_context(
                    tc.tile_pool(name="ops", bufs=2, space="PSUM")
                )
                p_pool = p4.enter_context(tc.tile_pool(name="p", bufs=6))
                den_pool = p4.enter_context(tc.tile_pool(name="den", bufs=3))
                rep2_pool = p4.enter_context(tc.tile_pool(name="rep2", bufs=4))
                out_pool = p4.enter_context(tc.tile_pool(name="osb", bufs=3))
                dram2 = p4.enter_context(
                    tc.tile_pool(name="dram2", bufs=4, space="DRAM")
                )
                wp_pool = p4.enter_context(tc.tile_pool(name="wp", bufs=1))
                y_pack = [
                    wp_pool.tile([128, T], F32R, tag=f"ypk{p}", name=f"ypk{p}")
                    for p in range(2)
                ]
                wproj_sb = []
                for p in range(2):
                    wt = wp_pool.tile([128, C], F32R, tag=f"wp{p}", name=f"wp{p}")
                    nc.sync.dma_start(
                        wt[:], d_wproj[128 * p : 128 * p + 128, :].bitcast(F32R)
                    )
                    wproj_sb.append(wt)

                for ci in range(NT):
                    isl = slice(NCH * ci, NCH * ci + NCH)
                    for h in range(HLOC):
                        p_pk, s_slot = divmod(h, 2)
                        yps = y_ps_pool.tile([65, NCH], F32, tag="yps", name="yps")
                        last_tj = 4 * ci + 3
                        for tj in range(4 * ci + 4):
                            sps = s_ps_pool.tile([128, NCH], F32, tag="sps", name="sps")
                            nc.tensor.matmul(
                                sps[:],
                                k_aug[h][:, 128 * tj : 128 * tj + 128],
                                q_aug[h][:, isl],
                                start=True,
                                stop=True,
                            )
                            r = tj - 4 * ci
                            if r < 0:
                                off = 0
                            else:
                                off = 128 * r
                                nc.vector.tensor_tensor(
                                    sps[:, off : off + 128],
                                    sps[:, off : off + 128],
                                    stair[:],
                                    ADD,
                                )
                            psb = p_pool.tile([128, NCH], BF16, tag="p", name="p")
                            nc.scalar.activation(
                                psb[:, off:NCH], sps[:, off:NCH], AF.Exp
                            )
                            nc.tensor.matmul(
                                yps[:, off:NCH],
                                v_sb[tj][:, 65 * h : 65 * h + 65],
                                psb[:, off:NCH],
                                start=(tj == 0),
                                stop=(tj == last_tj),
                                skip_group_check=True,
                            )
                        # normalize: 1/denom = exp(-ln(denom))
                        d1 = den_pool.tile([65, NCH], F32, tag="d1", name="d1")
                        nc.scalar.activation(d1[64:65, :], yps[64:65, :], AF.Ln)
                        nc.scalar.activation(
                            d1[64:65, :],
                            d1[64:65, :],
                            AF.Exp,
                            scale=neg1_col[64:65, :],
                        )
                        drow = dram2.tile([1, NCH], F32, tag="dden", name="dden")
                        nc.sync.dma_start(drow[:], d1[64:65, :])
                        rrep = rep2_pool.tile([64, NCH], F32, tag="rrep", name="rrep")
                        nc.sync.dma_start(
                            rrep[:], drow[0:1, :].partition_broadcast(64)
                        )
                        nc.vector.tensor_tensor(
                            y_pack[p_pk][64 * s_slot : 64 * s_slot + 64, isl],
                            yps[0:64, :],
                            rrep[:],
                            MUL,
                        )
                    # P5: projection for the 4 t-tiles covered by this chunk
                    for tt in range(4 * ci, 4 * ci + 4):
                        for cn in range(2):
                            osl = slice(512 * cn, 512 * cn + 512)
                            ops = o_ps_pool.tile([128, 512], F32, tag="ops", name="ops")
                            for p in range(2):
                                nc.tensor.matmul(
                                    ops[:],
                                    y_pack[p][:, 128 * tt : 128 * tt + 128],
                                    wproj_sb[p][:, osl],
                                    start=(p == 0),
                                    stop=(p == 1),
                                )
                            osb = out_pool.tile([128, 512], F32, tag="osb", name="osb")
                            nc.vector.tensor_copy(osb[:], ops[:])
                            nc.sync.dma_start(
                                d_out[128 * tt : 128 * tt + 128, osl], osb[:]
                            )

    split_excess_waits(nc, max_waits=1)
    return nc


def _host_shards(inputs):
    x = np.asarray(inputs["x"], np.float32)
    Wq = np.asarray(inputs["Wq"], np.float32)
    Wk = np.asarray(inputs["Wk"], np.float32)
    Wv = np.asarray(inputs["Wv"], np.float32)
    Wproj = np.asarray(inputs["Wproj"], np.float32)
    q_rms_w = np.asarray(inputs["q_rms_w"], np.float32)
    k_rms_w = np.asarray(inputs["k_rms_w"], np.float32)
    omega = np.asarray(inputs["omega"], np.float32)
    u = np.asarray(inputs["u"], np.float32)

    import ml_dtypes

    slopes = np.asarray(_get_alibi_slopes(H), np.float32)
    omega_eff = np.log1p(np.exp(omega)) * slopes  # softplus(omega) * slopes
    u_n = u / np.maximum(
        np.linalg.norm(u, axis=-1, keepdims=True), U_L2_EPS
    )
    sqrt_d = math.sqrt(D)

    iota = np.arange(T, dtype=np.float32)[None, :]
    ones_row = np.ones((1, T), np.float32)
    ones4 = np.ones((128, HLOC), np.float32).astype(ml_dtypes.bfloat16)
    jj = np.arange(128, dtype=np.float32)
    stair = np.where(jj[None, :] >= jj[:, None], 0.0, NEG_BIG).astype(np.float32)
    ssqw = np.zeros((128, 2), np.float32)
    ssqw[0:64, 0] = 1.0 / D
    ssqw[64:128, 1] = 1.0 / D
    qw8 = np.tile(q_rms_w / (8.0), 2)[:, None].astype(np.float32)
    kw = np.tile(k_rms_w, 2)[:, None].astype(np.float32)

    in_maps = []
    for core in range(8):
        b, g = divmod(core, HLOC)
        hs = slice(HLOC * g, HLOC * g + HLOC)
        cs = slice(HD * g, HD * g + HD)
        in_maps.append(
            {
                "xT": np.ascontiguousarray(x[b].T),
                "wq": np.ascontiguousarray(Wq[:, cs]),
                "wk": np.ascontiguousarray(Wk[:, cs]),
                "wv": np.ascontiguousarray(Wv[:, cs]),
                "wproj": np.ascontiguousarray(Wproj[cs, :]),
                "ucol": np.ascontiguousarray(u_n[hs].T / sqrt_d),
                "omg": np.ascontiguousarray(omega_eff[hs][:, None]),
                "negomg": np.ascontiguousarray(-omega_eff[hs][:, None]),
                "iota4": np.tile(iota, (HLOC, 1)),
                "oh16": np.eye(HLOC, dtype=np.float32).reshape(1, 16),
                "iota": iota,
                "ones_row": ones_row,
                "ones4": ones4,
                "stair": stair,
                "ssqw": ssqw,
                "qw8": qw8,
                "kw": kw,
            }
        )
    return in_maps


def kernel(**inputs):
    from concourse.bass_utils import run_bass_kernel_spmd

    if "nc" not in _cache:
        _cache["nc"] = _build_program()
    nc = _cache["nc"]

    in_maps = _host_shards(inputs)
    res = run_bass_kernel_spmd(nc, in_maps, core_ids=list(range(8)))
    out = np.zeros((B, T, C), np.float32)
    for core in range(8):
        b = core // HLOC
        out[b] += res.results[core]["out"]
    return out



# revision 10
# speedup vs baseline: 1.6291x; 1.6291x over previous
"""Causal self-attention (RMSNorm QK, key-gated ALiBi bias) on 8 TRN2 cores.

Sharding: data-parallel over batch (2) x tensor-parallel over heads (4 groups
of 4 heads) = 8 cores. Each core computes a partial c_proj output for its
batch; the host sums the 4 head-group partials per batch.

Device kernel v2 (restructured from the 347us baseline):
  - Inputs stream in as a few large rearranged DMAs; QKV matmuls start on
    the first T-chunk while later chunks load (kills the DMA-only lead-in).
  - RMS rsqrt batched: sum-of-squares rows for all 4 (pack, q/k) combos are
    stacked into one [8,512] PSUM tile by matmul, one Ln + one Exp per chunk.
  - rsqrt/denominator broadcasts over 64 partitions via PE selector matmuls
    (no DRAM roundtrips).
  - Key-gate softplus batched: gate logits for 4 heads stacked by matmul
    accumulation, one Exp + one Ln per chunk.
  - Bias rows (a_hi/a_lo/w_hi/w_lo) stacked head-major by matmul, one DVE
    copy + 4 DMAs per chunk.
  - Softmax denominator reciprocal on DVE (vector.reciprocal).
  - Causal stair mask folded into the score matmul accumulation group as a
    constant bf16 matmul (stairT^T @ I).
"""

import sys

if "/opt/trn_rl_repo" not in sys.path:
    sys.path.insert(0, "/opt/trn_rl_repo")

import math

import numpy as np

B, T, C = 2, 2048, 1024
H, D = 16, 64
HLOC = 4           # heads per core
HD = HLOC * D      # 256
NCH = 512          # T-chunk width
NT = T // NCH      # 4 chunks
JT = T // 128      # 16 j-tiles
KC = C // 128      # 8 contraction chunks
EPS_RMS = 1e-5
U_L2_EPS = 1e-6
NEG_BIG = -1.0e30

_cache = {}


def _get_alibi_slopes(n_heads):
    def pow2(n):
        start = 2 ** (-(2 ** (-(math.log2(n) - 3))))
        return [start * start**i for i in range(n)]

    if math.log2(n_heads).is_integer():
        return pow2(n_heads)
    c = 2 ** math.floor(math.log2(n_heads))
    s = pow2(c)
    extra = _get_alibi_slopes(2 * c)
    return s + extra[0::2][: n_heads - c]


def _build_program():
    import concourse.bass as bass
    import concourse.mybir as mybir
    import concourse.tile as tile
    from concourse.alu_op_type import AluOpType
    from concourse.vector_clock import ScopedClock

    F32 = mybir.dt.float32
    F32R = mybir.dt.float32r
    BF16 = mybir.dt.bfloat16
    AF = mybir.ActivationFunctionType
    MUL = AluOpType.mult
    SUB = AluOpType.subtract

    class PatchedTileContext(tile.TileContext):
        """Tail drain split into nops carrying <=2 sem waits each (this
        walrus build rejects CTRL instructions with more)."""

        def _drain_and_barrier(self, tick_clock, wait_clock):
            nc = self.nc
            probe = nc.sync.nop(nofuse=True)
            wait_clock.add_sem_waits(
                probe.ins, ScopedClock({None: tick_clock.global_clock})
            )
            si = probe.ins.sync_info
            waits = list(si.on_wait or []) if si is not None else []
            if len(waits) > 2:
                si.on_wait = waits[:2]
                rest = waits[2:]
                for i in range(0, len(rest), 2):
                    extra = nc.sync.nop(nofuse=True)
                    esi = extra.ins.sync_info
                    chunk = rest[i : i + 2]
                    if esi is None:
                        extra.ins.sync_info = mybir.SyncInfo(
                            on_wait=chunk, on_update=[]
                        )
                    else:
                        esi.on_wait = (esi.on_wait or []) + chunk
            nc.sync.drain()
            nc.all_engine_barrier()
            assert self.sems is not None
            popped = nc._tile_sem_poison_stack.pop()
            assert popped is self._sem_poison
            nc.clear_and_free_semaphores(list(self.sems.allocated().values()))
            nc.all_engine_barrier()

    def split_excess_waits(nc, max_waits=1):
        for f in nc.m.functions:
            for blk in f.blocks:
                new_insts = []
                for inst in blk.instructions:
                    si = inst.sync_info
                    if si is not None and si.on_wait and len(si.on_wait) > max_waits:
                        waits = list(si.on_wait)
                        si.on_wait = waits[-max_waits:]
                        rest = waits[:-max_waits]
                        for i in range(0, len(rest), max_waits):
                            nop = mybir.InstNoOp(
                                name=f"I-waitsplit-{nc.next_id()}",
                                ins=[],
                                outs=[],
                                engine=inst.engine,
                                sync_info=mybir.SyncInfo(
                                    on_wait=rest[i : i + max_waits], on_update=[]
                                ),
                            )
                            nc.register_instruction(nop)
                            new_insts.append(nop)
                    new_insts.append(inst)
                blk.instructions = new_insts

    nc = bass.Bass(trn_type="TRN2", num_devices=8, debug=False)

    # ---- DRAM I/O (per-core shards supplied by the host) ----
    d_xT = nc.dram_tensor("xT", [C, T], F32, kind="ExternalInput")
    d_wq = nc.dram_tensor("wq", [C, HD], F32, kind="ExternalInput")
    d_wk = nc.dram_tensor("wk", [C, HD], F32, kind="ExternalInput")
    d_wv = nc.dram_tensor("wv", [C, HD], F32, kind="ExternalInput")
    d_wproj = nc.dram_tensor("wproj", [HD, C], F32, kind="ExternalInput")
    d_ucolblk = nc.dram_tensor("ucolblk", [D, 16], F32, kind="ExternalInput")
    d_omg = nc.dram_tensor("omg", [HLOC, 1], F32, kind="ExternalInput")
    d_negomg = nc.dram_tensor("negomg", [HLOC, 1], F32, kind="ExternalInput")
    d_iota4 = nc.dram_tensor("iota4", [HLOC, T], F32, kind="ExternalInput")
    d_qrows = nc.dram_tensor("qrows", [4, T], F32, kind="ExternalInput")
    d_ones4 = nc.dram_tensor("ones4", [128, HLOC], BF16, kind="ExternalInput")
    d_stairT = nc.dram_tensor("stairT", [128, 128], BF16, kind="ExternalInput")
    d_ident = nc.dram_tensor("ident", [128, 128], BF16, kind="ExternalInput")
    d_selq = nc.dram_tensor("selq", [4, 256], F32, kind="ExternalInput")
    d_scat = nc.dram_tensor("scat", [4, 64], F32, kind="ExternalInput")
    d_ssqw4 = nc.dram_tensor("ssqw4", [128, 8], F32, kind="ExternalInput")
    d_ones64 = nc.dram_tensor("ones64", [1, 64], F32, kind="ExternalInput")
    d_qw8 = nc.dram_tensor("qw8", [128, 1], F32, kind="ExternalInput")
    d_kw = nc.dram_tensor("kw", [128, 1], F32, kind="ExternalInput")
    d_out = nc.dram_tensor("out", [T, C], F32, kind="ExternalOutput")

    with PatchedTileContext(nc) as tc:
        from contextlib import ExitStack

        with ExitStack() as top:
            persist = top.enter_context(tc.tile_pool(name="persist", bufs=1))

            # ---- persistent SBUF tensors ----
            q_aug = [persist.tile([68, T], F32R, tag=f"qaug{h}", name=f"qaug{h}") for h in range(HLOC)]
            k_aug = [persist.tile([68, T], F32R, tag=f"kaug{h}", name=f"kaug{h}") for h in range(HLOC)]
            v_sb = [
                persist.tile([128, HLOC * 65], BF16, tag=f"vsb{t}", name=f"vsb{t}") for t in range(JT)
            ]
            y_pack = [
                persist.tile([128, T], F32R, tag=f"ypk{p}", name=f"ypk{p}")
                for p in range(2)
            ]

            # ---- weights: one rearranged DMA each ----
            wq_sb = persist.tile([128, 2048], F32R, tag="wq", name="wq")
            wk_sb = persist.tile([128, 2048], F32R, tag="wk", name="wk")
            wv_sb = persist.tile([128, 2048], F32R, tag="wv", name="wv")
            wproj_sb = persist.tile([128, 2048], F32R, tag="wproj", name="wproj")
            for wsb, dten in ((wq_sb, d_wq), (wk_sb, d_wk)):
                nc.sync.dma_start(
                    wsb[:].rearrange("p (c j) -> p c j", c=KC),
                    dten[:].bitcast(F32R).rearrange("(c p) j -> p c j", p=128),
                )

            # ---- x chunks: 2 DMAs per T-chunk (4 contraction chunks each) ----
            xpool = top.enter_context(tc.tile_pool(name="xT", bufs=1))

            def load_xn(n):
                sl = slice(NCH * n, NCH * n + NCH)
                xt = xpool.tile([128, 4096], F32R, tag=f"x{n % 2}", name=f"x{n}")
                for half in range(2):
                    src = (
                        d_xT[512 * half : 512 * half + 512, sl]
                        .bitcast(F32R)
                        .rearrange("(c p) t -> p c t", p=128)
                    )
                    dst = xt[:, 2048 * half : 2048 * half + 2048].rearrange(
                        "p (c t) -> p c t", c=4
                    )
                    nc.sync.dma_start(dst, src)
                return xt

            x_tiles = {0: load_xn(0)}

            # remaining weights / consts (after the first x chunk is queued)
            nc.sync.dma_start(
                wv_sb[:].rearrange("p (c j) -> p c j", c=KC),
                d_wv[:].bitcast(F32R).rearrange("(c p) j -> p c j", p=128),
            )
            nc.sync.dma_start(
                wproj_sb[:].rearrange("p (g j) -> p g j", g=2),
                d_wproj[:].bitcast(F32R).rearrange("(g p) j -> p g j", p=128),
            )

            stairT = persist.tile([128, 128], BF16, tag="stairT", name="stairT")
            nc.sync.dma_start(stairT[:], d_stairT[:])
            ident = persist.tile([128, 128], BF16, tag="ident", name="ident")
            nc.sync.dma_start(ident[:], d_ident[:])
            selq = persist.tile([4, 256], F32R, tag="selq", name="selq")
            nc.sync.dma_start(selq[:], d_selq[:].bitcast(F32R))
            scat = persist.tile([4, 64], F32R, tag="scat", name="scat")
            nc.sync.dma_start(scat[:], d_scat[:].bitcast(F32R))
            ssqw4 = persist.tile([128, 8], F32R, tag="ssqw4", name="ssqw4")
            nc.sync.dma_start(ssqw4[:], d_ssqw4[:].bitcast(F32R))
            ucolblk = persist.tile([D, 16], F32R, tag="ucolblk", name="ucolblk")
            nc.sync.dma_start(ucolblk[:], d_ucolblk[:].bitcast(F32R))
            ones64 = persist.tile([1, 64], F32R, tag="ones64", name="ones64")
            nc.sync.dma_start(ones64[:], d_ones64[:].bitcast(F32R))
            omg = persist.tile([HLOC, 1], F32, tag="omg", name="omg")
            nc.sync.dma_start(omg[:], d_omg[:])
            negomg = persist.tile([HLOC, 1], F32, tag="negomg", name="negomg")
            nc.sync.dma_start(negomg[:], d_negomg[:])
            qw8 = persist.tile([128, 1], F32, tag="qw8", name="qw8")
            nc.sync.dma_start(qw8[:], d_qw8[:])
            kw = persist.tile([128, 1], F32, tag="kw", name="kw")
            nc.sync.dma_start(kw[:], d_kw[:])

            # q_aug fixed rows 64:68 = ones, ones, iota, iota
            for h in range(HLOC):
                nc.sync.dma_start(q_aug[h][64:68, :], d_qrows[:].bitcast(F32R))
            # v ones columns
            for t in range(JT):
                dst = v_sb[t][:].rearrange("p (h d) -> p h d", h=HLOC)[:, :, 64:65]
                nc.sync.dma_start(dst, d_ones4[:].rearrange("p (h o) -> p h o", o=1))

            eps8 = persist.tile([8, 1], F32, tag="eps8", name="eps8")
            nc.vector.memset(eps8[:], EPS_RMS)
            neghalf8 = persist.tile([8, 1], F32, tag="neghalf8", name="neghalf8")
            nc.vector.memset(neghalf8[:], -0.5)
            one4 = persist.tile([4, 1], F32, tag="one4", name="one4")
            nc.vector.memset(one4[:], 1.0)

            # ================= P2+P3: QKV, rms, gate, bias rows ============
            with ExitStack() as p2:
                qk_ps = p2.enter_context(
                    tc.tile_pool(name="qkps", bufs=3, space="PSUM")
                )
                v_ps = p2.enter_context(
                    tc.tile_pool(name="vps", bufs=2, space="PSUM")
                )
                s8_ps = p2.enter_context(
                    tc.tile_pool(name="s8ps", bufs=1, space="PSUM")
                )
                st_ps = p2.enter_context(
                    tc.tile_pool(name="stps", bufs=1, space="PSUM")
                )
                rep_ps = p2.enter_context(
                    tc.tile_pool(name="repps", bufs=1, space="PSUM")
                )
                sq_pool = p2.enter_context(tc.tile_pool(name="qsq", bufs=2))
                rep_sb = p2.enter_context(tc.tile_pool(name="repS", bufs=2))
                rsq_pool = p2.enter_context(tc.tile_pool(name="rsq", bufs=2))
                g_pool = p2.enter_context(tc.tile_pool(name="gate", bufs=1))
                st_pool = p2.enter_context(tc.tile_pool(name="stsb", bufs=2))

                def qk_batch(n, p, xt, sl):
                    """One head-pack p: q+k projection, batched rsqrt, returns
                    the two PSUM tiles + the f32r rsq rows for rep matmuls."""
                    ps_list = []
                    qsq_list = []
                    s4 = s8_ps.tile([4, NCH], F32, tag="s8", name=f"s4_{n}_{p}")
                    for loc in range(2):  # 0 = q, 1 = k
                        wsb = wk_sb if loc else wq_sb
                        ps = qk_ps.tile([128, NCH], F32, tag="qk", name="qk")
                        for cc in range(KC):
                            nc.tensor.matmul(
                                ps[:],
                                wsb[:, 256 * cc + 128 * p : 256 * cc + 128 * p + 128],
                                xt[:, 512 * cc : 512 * cc + 512],
                                start=(cc == 0),
                                stop=(cc == KC - 1),
                            )
                        ps_list.append(ps)
                        qsq = sq_pool.tile([128, NCH], F32R, tag="qsq", name="qsq")
                        nc.scalar.activation(qsq[:], ps[:], AF.Square)
                        qsq_list.append(qsq)
                    for loc, qsq in enumerate(qsq_list):
                        nc.tensor.matmul(
                            s4[:],
                            ssqw4[:, 4 * loc : 4 * loc + 4],
                            qsq[:],
                            start=(loc == 0),
                            stop=(loc == 1),
                            skip_group_check=True,
                        )
                    rsq_f = rsq_pool.tile([4, NCH], F32, tag="rsqf", name="rsqf")
                    nc.scalar.activation(rsq_f[:], s4[:], AF.Ln, bias=eps8[0:4, :])
                    rsq = rsq_pool.tile([4, NCH], F32R, tag="rsq", name="rsq")
                    nc.scalar.activation(
                        rsq[:], rsq_f[:], AF.Exp, scale=neghalf8[0:4, :]
                    )
                    return ps_list, rsq

                def rms_apply(p, ps_list, rsq, sl):
                    for loc, ps in enumerate(ps_list):
                        rep = rep_ps.tile([128, NCH], F32, tag="rep", name="rep")
                        nc.tensor.matmul(
                            rep[:],
                            selq[:, 128 * loc : 128 * loc + 128],
                            rsq[:],
                            start=True,
                            stop=True,
                        )
                        repS = rep_sb.tile([128, NCH], F32, tag="repS", name="repS")
                        nc.scalar.copy(repS[:], rep[:])
                        wcol = kw if loc else qw8
                        aug_set = k_aug if loc else q_aug
                        for s in range(2):
                            nc.vector.scalar_tensor_tensor(
                                aug_set[2 * p + s][0:64, sl],
                                ps[64 * s : 64 * s + 64, :],
                                wcol[64 * s : 64 * s + 64, :],
                                repS[64 * s : 64 * s + 64, :],
                                MUL,
                                MUL,
                            )

                def v_group(n, tl, xt):
                    t = 4 * n + tl
                    vps = v_ps.tile([128, NCH], F32, tag="vq", name="vps")
                    for cc in range(KC):
                        nc.tensor.matmul(
                            vps[:, 0:HD],
                            xt[:, 512 * cc + 128 * tl : 512 * cc + 128 * tl + 128],
                            wv_sb[:, 256 * cc : 256 * cc + 256],
                            start=(cc == 0),
                            stop=(cc == KC - 1),
                        )
                    dst = v_sb[t][:].rearrange("p (h d) -> p h d", h=HLOC)[:, :, 0:64]
                    nc.scalar.copy(
                        dst, vps[:, 0:HD].rearrange("p (h d) -> p h d", h=HLOC)
                    )

                for n in range(NT):
                    sl = slice(NCH * n, NCH * n + NCH)
                    if n + 1 < NT:
                        x_tiles[n + 1] = load_xn(n + 1)
                    xt = x_tiles.pop(n)

                    psA, rsqA = qk_batch(n, 0, xt, sl)
                    v_group(n, 0, xt)  # PE busy while ACT does batch-A rsqrt
                    v_group(n, 1, xt)
                    rms_apply(0, psA, rsqA, sl)
                    psB, rsqB = qk_batch(n, 1, xt, sl)
                    v_group(n, 2, xt)
                    v_group(n, 3, xt)
                    rms_apply(1, psB, rsqB, sl)

                    # --- P3: key gate + bias rows ---
                    g4 = s8_ps.tile([8, NCH], F32, tag="s8", name="g4")
                    for h in range(HLOC):
                        nc.tensor.matmul(
                            g4[0:4, :],
                            ucolblk[:, 4 * h : 4 * h + 4],
                            k_aug[h][0:64, sl],
                            start=(h == 0),
                            stop=(h == HLOC - 1),
                            skip_group_check=True,
                        )
                    gsc = g_pool.tile([4, NCH], F32, tag="gsc", name="gsc")
                    nc.scalar.activation(gsc[:], g4[0:4, :], AF.Exp)
                    gate4 = g_pool.tile([4, NCH], F32, tag="gate4", name="gate4")
                    nc.scalar.activation(gate4[:], gsc[:], AF.Ln, bias=one4[:])
                    iot = g_pool.tile([4, NCH], F32, tag="iot", name="iot")
                    nc.sync.dma_start(iot[:], d_iota4[:, sl])
                    a4f = g_pool.tile([4, NCH], F32, tag="a4f", name="a4f")
                    nc.vector.scalar_tensor_tensor(
                        a4f[:], gate4[:], omg[:], iot[:], MUL, MUL
                    )
                    w4f = g_pool.tile([4, NCH], F32, tag="w4f", name="w4f")
                    nc.vector.tensor_scalar_mul(w4f[:], gate4[:], negomg[:])
                    a_hi = g_pool.tile([4, NCH], F32R, tag="a_hi", name="a_hi")
                    nc.vector.tensor_copy(a_hi[:], a4f[:])
                    a_lo = g_pool.tile([4, NCH], F32R, tag="a_lo", name="a_lo")
                    nc.vector.scalar_tensor_tensor(
                        a_lo[:], a4f[:], 1.0, a_hi[:].bitcast(F32), MUL, SUB
                    )
                    w_hi = g_pool.tile([4, NCH], F32R, tag="w_hi", name="w_hi")
                    nc.vector.tensor_copy(w_hi[:], w4f[:])
                    w_lo = g_pool.tile([4, NCH], F32R, tag="w_lo", name="w_lo")
                    nc.vector.scalar_tensor_tensor(
                        w_lo[:], w4f[:], 1.0, w_hi[:].bitcast(F32), MUL, SUB
                    )
                    # stack head-major: st[4h+r] = S_r[h]
                    st16 = st_ps.tile([16, NCH], F32, tag="st16", name="st16")
                    for r, src in enumerate((a_hi, a_lo, w_hi, w_lo)):
                        nc.tensor.matmul(
                            st16[:],
                            scat[:, 16 * r : 16 * r + 16],
                            src[:],
                            start=(r == 0),
                            stop=(r == 3),
                            skip_group_check=True,
                        )
                    stsb = st_pool.tile([16, NCH], F32R, tag="stsb", name="stsb")
                    nc.vector.tensor_copy(stsb[:], st16[:])
                    for h in range(HLOC):
                        nc.sync.dma_start(
                            k_aug[h][64:68, sl], stsb[4 * h : 4 * h + 4, :]
                        )

            # ================= P4 + P5: attention & projection =============
            with ExitStack() as p4:
                s_ps_pool = p4.enter_context(
                    tc.tile_pool(name="sps", bufs=3, space="PSUM")
                )
                y_ps_pool = p4.enter_context(
                    tc.tile_pool(name="yps", bufs=2, space="PSUM")
                )
                r_ps_pool = p4.enter_context(
                    tc.tile_pool(name="rps", bufs=1, space="PSUM")
                )
                o_ps_pool = p4.enter_context(
                    tc.tile_pool(name="ops", bufs=2, space="PSUM")
                )
                p_pool = p4.enter_context(tc.tile_pool(name="p", bufs=6))
                rcp_pool = p4.enter_context(tc.tile_pool(name="rcp", bufs=2))
                rep4_sb = p4.enter_context(tc.tile_pool(name="rep4", bufs=2))
                out_pool = p4.enter_context(tc.tile_pool(name="osb", bufs=2))

                for ci in range(NT):
                    isl = slice(NCH * ci, NCH * ci + NCH)
                    for h in range(HLOC):
                        p_pk, s_slot = divmod(h, 2)
                        yps = y_ps_pool.tile([65, NCH], F32, tag="yps", name="yps")
                        last_tj = 4 * ci + 3
                        for tj in range(4 * ci + 4):
                            r = tj - 4 * ci
                            off = 0 if r < 0 else 128 * r
                            smt = min(off, 256)
                            sps = s_ps_pool.tile([128, NCH], F32, tag="sps", name="sps")
                            nc.tensor.matmul(
                                sps[:, smt:NCH],
                                k_aug[h][:, 128 * tj : 128 * tj + 128],
                                q_aug[h][:, NCH * ci + smt : NCH * ci + NCH],
                                start=True,
                                stop=(r < 0),
                                skip_group_check=True,
                            )
                            if r >= 0:
                                # causal stair mask via constant matmul
                                nc.tensor.matmul(
                                    sps[:, off : off + 128],
                                    stairT[:],
                                    ident[:],
                                    start=False,
                                    stop=True,
                                    skip_group_check=True,
                                )
                            psb = p_pool.tile([128, NCH], BF16, tag="p", name="p")
                            nc.scalar.activation(
                                psb[:, off:NCH], sps[:, off:NCH], AF.Exp
                            )
                            nc.tensor.matmul(
                                yps[:, off:NCH],
                                v_sb[tj][:, 65 * h : 65 * h + 65],
                                psb[:, off:NCH],
                                start=(tj == 0),
                                stop=(tj == last_tj),
                                skip_group_check=True,
                            )
                        # normalize via DVE reciprocal + PE broadcast
                        rcp = rcp_pool.tile([1, NCH], F32R, tag="rcp", name="rcp")
                        with nc.allow_low_precision(reason="softmax denom bcast"):
                            nc.vector.reciprocal(rcp[:], yps[64:65, :])
                        rep = r_ps_pool.tile([64, NCH], F32, tag="rep", name="rep")
                        nc.tensor.matmul(
                            rep[:], ones64[:], rcp[:], start=True, stop=True
                        )
                        repS = rep4_sb.tile([64, NCH], F32, tag="rep4", name="rep4")
                        nc.vector.tensor_copy(repS[:], rep[:])
                        nc.vector.tensor_tensor(
                            y_pack[p_pk][64 * s_slot : 64 * s_slot + 64, isl],
                            yps[0:64, :],
                            repS[:],
                            MUL,
                        )
                    # P5: projection for the 4 t-tiles covered by this chunk
                    for tt in range(4 * ci, 4 * ci + 4):
                        osb = out_pool.tile([128, 1024], F32, tag="osb", name="osb")
                        for cn in range(2):
                            osl = slice(512 * cn, 512 * cn + 512)
                            ops = o_ps_pool.tile([128, 512], F32, tag="ops", name="ops")
                            for p in range(2):
                                nc.tensor.matmul(
                                    ops[:],
                                    y_pack[p][:, 128 * tt : 128 * tt + 128],
                                    wproj_sb[:, 1024 * p + 512 * cn : 1024 * p + 512 * cn + 512],
                                    start=(p == 0),
                                    stop=(p == 1),
                                )
                            nc.vector.tensor_copy(osb[:, osl], ops[:])
                        nc.sync.dma_start(
                            d_out[128 * tt : 128 * tt + 128, :], osb[:]
                        )

    split_excess_waits(nc, max_waits=1)
    return nc


def _host_shards(inputs):
    x = np.asarray(inputs["x"], np.float32)
    Wq = np.asarray(inputs["Wq"], np.float32)
    Wk = np.asarray(inputs["Wk"], np.float32)
    Wv = np.asarray(inputs["Wv"], np.float32)
    Wproj = np.asarray(inputs["Wproj"], np.float32)
    q_rms_w = np.asarray(inputs["q_rms_w"], np.float32)
    k_rms_w = np.asarray(inputs["k_rms_w"], np.float32)
    omega = np.asarray(inputs["omega"], np.float32)
    u = np.asarray(inputs["u"], np.float32)

    import ml_dtypes

    slopes = np.asarray(_get_alibi_slopes(H), np.float32)
    omega_eff = np.log1p(np.exp(omega)) * slopes  # softplus(omega) * slopes
    u_n = u / np.maximum(np.linalg.norm(u, axis=-1, keepdims=True), U_L2_EPS)
    sqrt_d = math.sqrt(D)

    iota = np.arange(T, dtype=np.float32)[None, :]
    qrows = np.concatenate(
        [np.ones((2, T), np.float32), np.tile(iota, (2, 1))], axis=0
    )
    ones4 = np.ones((128, HLOC), np.float32).astype(ml_dtypes.bfloat16)
    jj = np.arange(128, dtype=np.float32)
    stair = np.where(jj[None, :] >= jj[:, None], 0.0, NEG_BIG).astype(np.float32)
    stairT = stair.T.astype(ml_dtypes.bfloat16)
    ident = np.eye(128, dtype=np.float32).astype(ml_dtypes.bfloat16)
    # ssqw4 [128, 8]: block loc (cols 4*loc..+4): col 4*loc + 2*loc + s <- 1/D
    # on rows 64s.. (s4 rows are 2*loc + s)
    ssqw4 = np.zeros((128, 8), np.float32)
    for loc in range(2):
        for s in range(2):
            ssqw4[64 * s : 64 * s + 64, 4 * loc + 2 * loc + s] = 1.0 / D
    # selq [4, 256]: block loc: selq[2*loc + (m>=64), 128*loc + m] = 1
    selq = np.zeros((4, 256), np.float32)
    for loc in range(2):
        for m in range(128):
            selq[2 * loc + (m >= 64), 128 * loc + m] = 1.0
    # scat [4, 64]: block r: scat[h, 16r + 4h + r] = 1
    scat = np.zeros((4, 64), np.float32)
    for r in range(4):
        for h in range(4):
            scat[h, 16 * r + 4 * h + r] = 1.0
    ones64 = np.ones((1, 64), np.float32)
    qw8 = np.tile(q_rms_w / 8.0, 2)[:, None].astype(np.float32)
    kw = np.tile(k_rms_w, 2)[:, None].astype(np.float32)

    in_maps = []
    for core in range(8):
        b, g = divmod(core, HLOC)
        hs = slice(HLOC * g, HLOC * g + HLOC)
        cs = slice(HD * g, HD * g + HD)
        # ucolblk [64, 16]: col 4h+j = u_n[head h]/sqrt(D) if j==h else 0
        ucolblk = np.zeros((D, 16), np.float32)
        for h in range(HLOC):
            ucolblk[:, 4 * h + h] = u_n[HLOC * g + h] / sqrt_d
        in_maps.append(
            {
                "xT": np.ascontiguousarray(x[b].T),
                "wq": np.ascontiguousarray(Wq[:, cs]),
                "wk": np.ascontiguousarray(Wk[:, cs]),
                "wv": np.ascontiguousarray(Wv[:, cs]),
                "wproj": np.ascontiguousarray(Wproj[cs, :]),
                "ucolblk": ucolblk,
                "omg": np.ascontiguousarray(omega_eff[hs][:, None]),
                "negomg": np.ascontiguousarray(-omega_eff[hs][:, None]),
                "iota4": np.tile(iota, (HLOC, 1)),
                "qrows": qrows,
                "ones4": ones4,
                "stairT": stairT,
                "ident": ident,
                "selq": selq,
                "scat": scat,
                "ssqw4": ssqw4,
                "ones64": ones64,
                "qw8": qw8,
                "kw": kw,
            }
        )
    return in_maps


def kernel(**inputs):
    from concourse.bass_utils import run_bass_kernel_spmd

    if "nc" not in _cache:
        _cache["nc"] = _build_program()
    nc = _cache["nc"]

    in_maps = _host_shards(inputs)
    res = run_bass_kernel_spmd(nc, in_maps, core_ids=list(range(8)))
    out = np.zeros((B, T, C), np.float32)
    for core in range(8):
        b = core // HLOC
        out[b] += res.results[core]["out"]
    return out


# revision 25
# speedup vs baseline: 1.7098x; 1.0496x over previous
"""Causal self-attention (RMSNorm QK, key-gated ALiBi bias) on 8 TRN2 cores.

Sharding: data-parallel over batch (2) x tensor-parallel over heads (4 groups
of 4 heads) = 8 cores. Each core computes a partial c_proj output for its
batch; the host sums the 4 head-group partials per batch.

Device kernel v2 (restructured from the 347us baseline):
  - Inputs stream in as a few large rearranged DMAs; QKV matmuls start on
    the first T-chunk while later chunks load (kills the DMA-only lead-in).
  - RMS rsqrt batched: sum-of-squares rows for all 4 (pack, q/k) combos are
    stacked into one [8,512] PSUM tile by matmul, one Ln + one Exp per chunk.
  - rsqrt/denominator broadcasts over 64 partitions via PE selector matmuls
    (no DRAM roundtrips).
  - Key-gate softplus batched: gate logits for 4 heads stacked by matmul
    accumulation, one Exp + one Ln per chunk.
  - Bias rows (a_hi/a_lo/w_hi/w_lo) stacked head-major by matmul, one DVE
    copy + 4 DMAs per chunk.
  - Softmax denominator reciprocal on DVE (vector.reciprocal).
  - Causal stair mask folded into the score matmul accumulation group as a
    constant bf16 matmul (stairT^T @ I).
"""

import sys

if "/opt/trn_rl_repo" not in sys.path:
    sys.path.insert(0, "/opt/trn_rl_repo")

import math

import numpy as np

B, T, C = 2, 2048, 1024
H, D = 16, 64
HLOC = 4           # heads per core
HD = HLOC * D      # 256
NCH = 512          # T-chunk width
NT = T // NCH      # 4 chunks
JT = T // 128      # 16 j-tiles
KC = C // 128      # 8 contraction chunks
EPS_RMS = 1e-5
U_L2_EPS = 1e-6
NEG_BIG = -1.0e30

_cache = {}

# P4 emission config (sweepable)
CFG = {
    "paired": True,
    "alternate": True,
    "sps_bufs": 2,
    "p_bufs": 6,
    "rep_own": False,
    "pv_defer": 6,
}


def _get_alibi_slopes(n_heads):
    def pow2(n):
        start = 2 ** (-(2 ** (-(math.log2(n) - 3))))
        return [start * start**i for i in range(n)]

    if math.log2(n_heads).is_integer():
        return pow2(n_heads)
    c = 2 ** math.floor(math.log2(n_heads))
    s = pow2(c)
    extra = _get_alibi_slopes(2 * c)
    return s + extra[0::2][: n_heads - c]


def _build_program(cfg=None):
    cfg = dict(CFG if cfg is None else cfg)
    import concourse.bass as bass
    import concourse.mybir as mybir
    import concourse.tile as tile
    from concourse.alu_op_type import AluOpType
    from concourse.vector_clock import ScopedClock

    F32 = mybir.dt.float32
    F32R = mybir.dt.float32r
    BF16 = mybir.dt.bfloat16
    AF = mybir.ActivationFunctionType
    MUL = AluOpType.mult
    SUB = AluOpType.subtract

    class PatchedTileContext(tile.TileContext):
        """Tail drain split into nops carrying <=2 sem waits each (this
        walrus build rejects CTRL instructions with more)."""

        def _drain_and_barrier(self, tick_clock, wait_clock):
            nc = self.nc
            probe = nc.sync.nop(nofuse=True)
            wait_clock.add_sem_waits(
                probe.ins, ScopedClock({None: tick_clock.global_clock})
            )
            si = probe.ins.sync_info
            waits = list(si.on_wait or []) if si is not None else []
            if len(waits) > 2:
                si.on_wait = waits[:2]
                rest = waits[2:]
                for i in range(0, len(rest), 2):
                    extra = nc.sync.nop(nofuse=True)
                    esi = extra.ins.sync_info
                    chunk = rest[i : i + 2]
                    if esi is None:
                        extra.ins.sync_info = mybir.SyncInfo(
                            on_wait=chunk, on_update=[]
                        )
                    else:
                        esi.on_wait = (esi.on_wait or []) + chunk
            nc.sync.drain()
            nc.all_engine_barrier()
            assert self.sems is not None
            popped = nc._tile_sem_poison_stack.pop()
            assert popped is self._sem_poison
            nc.clear_and_free_semaphores(list(self.sems.allocated().values()))
            nc.all_engine_barrier()

    def split_excess_waits(nc, max_waits=1):
        for f in nc.m.functions:
            for blk in f.blocks:
                new_insts = []
                for inst in blk.instructions:
                    si = inst.sync_info
                    if si is not None and si.on_wait and len(si.on_wait) > max_waits:
                        waits = list(si.on_wait)
                        si.on_wait = waits[-max_waits:]
                        rest = waits[:-max_waits]
                        for i in range(0, len(rest), max_waits):
                            nop = mybir.InstNoOp(
                                name=f"I-waitsplit-{nc.next_id()}",
                                ins=[],
                                outs=[],
                                engine=inst.engine,
                                sync_info=mybir.SyncInfo(
                                    on_wait=rest[i : i + max_waits], on_update=[]
                                ),
                            )
                            nc.register_instruction(nop)
                            new_insts.append(nop)
                    new_insts.append(inst)
                blk.instructions = new_insts

    nc = bass.Bass(trn_type="TRN2", num_devices=8, debug=False)

    # ---- DRAM I/O (per-core shards supplied by the host) ----
    d_xT = nc.dram_tensor("xT", [C, T], F32, kind="ExternalInput")
    d_wq = nc.dram_tensor("wq", [C, HD], F32, kind="ExternalInput")
    d_wk = nc.dram_tensor("wk", [C, HD], F32, kind="ExternalInput")
    d_wv = nc.dram_tensor("wv", [C, HD], F32, kind="ExternalInput")
    d_wproj = nc.dram_tensor("wproj", [HD, C], F32, kind="ExternalInput")
    d_ucolblk = nc.dram_tensor("ucolblk", [D, 16], F32, kind="ExternalInput")
    d_omg = nc.dram_tensor("omg", [HLOC, 1], F32, kind="ExternalInput")
    d_negomg = nc.dram_tensor("negomg", [HLOC, 1], F32, kind="ExternalInput")
    d_iota4 = nc.dram_tensor("iota4", [HLOC, T], F32, kind="ExternalInput")
    d_qrows = nc.dram_tensor("qrows", [4, T], F32, kind="ExternalInput")
    d_ones64c = nc.dram_tensor("ones64c", [128, JT * HLOC], BF16, kind="ExternalInput")
    d_stairT = nc.dram_tensor("stairT", [128, 128], BF16, kind="ExternalInput")
    d_ident = nc.dram_tensor("ident", [128, 128], BF16, kind="ExternalInput")
    d_scat = nc.dram_tensor("scat", [4, 64], F32, kind="ExternalInput")
    d_selq = nc.dram_tensor("selq", [4, 256], F32, kind="ExternalInput")
    d_ones64 = nc.dram_tensor("ones64", [1, 64], F32, kind="ExternalInput")
    d_ssqw4 = nc.dram_tensor("ssqw4", [128, 8], F32, kind="ExternalInput")
    d_qw8 = nc.dram_tensor("qw8", [128, 1], F32, kind="ExternalInput")
    d_kw = nc.dram_tensor("kw", [128, 1], F32, kind="ExternalInput")
    d_out = nc.dram_tensor("out", [T, C], F32, kind="ExternalOutput")

    with PatchedTileContext(nc) as tc:
        from contextlib import ExitStack

        with ExitStack() as top:
            persist = top.enter_context(tc.tile_pool(name="persist", bufs=1))

            # ---- persistent SBUF tensors ----
            q_aug = [persist.tile([68, T], F32R, tag=f"qaug{h}", name=f"qaug{h}") for h in range(HLOC)]
            k_aug = [persist.tile([68, T], F32R, tag=f"kaug{h}", name=f"kaug{h}") for h in range(HLOC)]
            vbig = persist.tile([128, JT * HLOC * 65], BF16, tag="vbig", name="vbig")
            v_sb = [vbig[:, 260 * t : 260 * t + 260] for t in range(JT)]
            y_pack = [
                persist.tile([128, T], F32R, tag=f"ypk{p}", name=f"ypk{p}")
                for p in range(2)
            ]

            # ---- weights: one rearranged DMA each ----
            wq_sb = persist.tile([128, 2048], F32R, tag="wq", name="wq")
            wk_sb = persist.tile([128, 2048], F32R, tag="wk", name="wk")
            wv_sb = persist.tile([128, 2048], F32R, tag="wv", name="wv")
            wproj_sb = persist.tile([128, 2048], F32R, tag="wproj", name="wproj")
            def load_w(wsb, dten, half=None):
                halves = range(2) if half is None else [half]
                for hf in halves:
                    nc.sync.dma_start(
                        wsb[:, 1024 * hf : 1024 * hf + 1024].rearrange(
                            "p (c j) -> p c j", c=KC // 2
                        ),
                        dten[512 * hf : 512 * hf + 512, :]
                        .bitcast(F32R)
                        .rearrange("(c p) j -> p c j", p=128),
                    )

            # ---- x chunks: 2 DMAs per T-chunk (4 contraction chunks each) ----
            xpool = top.enter_context(tc.tile_pool(name="xT", bufs=1))

            def load_xn_half(xt, n, hf):
                sl = slice(NCH * n, NCH * n + NCH)
                src = (
                    d_xT[512 * hf : 512 * hf + 512, sl]
                    .bitcast(F32R)
                    .rearrange("(c p) t -> p c t", p=128)
                )
                dst = xt[:, 2048 * hf : 2048 * hf + 2048].rearrange(
                    "p (c t) -> p c t", c=4
                )
                nc.sync.dma_start(dst, src)

            def load_xn(n):
                xt = xpool.tile([128, 4096], F32R, tag=f"x{n % 2}", name=f"x{n}")
                load_xn_half(xt, n, 0)
                load_xn_half(xt, n, 1)
                return xt

            # interleave the first x chunk with the q/k weights so the first
            # projection matmuls can start as early as possible; x1 right
            # after wk so chunk n=1 is never starved behind const DMAs
            load_w(wq_sb, d_wq, half=0)
            x0 = xpool.tile([128, 4096], F32R, tag="x0", name="x_0")
            load_xn_half(x0, 0, 0)
            load_w(wq_sb, d_wq, half=1)
            load_xn_half(x0, 0, 1)
            x_tiles = {0: x0}
            load_w(wk_sb, d_wk)
            x_tiles[1] = load_xn(1)
            load_w(wv_sb, d_wv)
            nc.sync.dma_start(
                wproj_sb[:].rearrange("p (g j) -> p g j", g=2),
                d_wproj[:].bitcast(F32R).rearrange("(g p) j -> p g j", p=128),
            )

            stairT = persist.tile([128, 128], BF16, tag="stairT", name="stairT")
            nc.sync.dma_start(stairT[:], d_stairT[:])
            ident = persist.tile([128, 128], BF16, tag="ident", name="ident")
            nc.sync.dma_start(ident[:], d_ident[:])
            scat = persist.tile([4, 64], F32R, tag="scat", name="scat")
            nc.sync.dma_start(scat[:], d_scat[:].bitcast(F32R))
            selq = persist.tile([4, 256], F32R, tag="selq", name="selq")
            nc.sync.dma_start(selq[:], d_selq[:].bitcast(F32R))
            ones64 = persist.tile([1, 64], F32R, tag="ones64", name="ones64")
            nc.sync.dma_start(ones64[:], d_ones64[:].bitcast(F32R))
            ssqw4 = persist.tile([128, 8], F32R, tag="ssqw4", name="ssqw4")
            nc.sync.dma_start(ssqw4[:], d_ssqw4[:].bitcast(F32R))
            ucolblk = persist.tile([D, 16], F32R, tag="ucolblk", name="ucolblk")
            nc.sync.dma_start(ucolblk[:], d_ucolblk[:].bitcast(F32R))
            omg = persist.tile([HLOC, 1], F32, tag="omg", name="omg")
            nc.sync.dma_start(omg[:], d_omg[:])
            negomg = persist.tile([HLOC, 1], F32, tag="negomg", name="negomg")
            nc.sync.dma_start(negomg[:], d_negomg[:])
            qw8 = persist.tile([128, 1], F32, tag="qw8", name="qw8")
            nc.sync.dma_start(qw8[:], d_qw8[:])
            kw = persist.tile([128, 1], F32, tag="kw", name="kw")
            nc.sync.dma_start(kw[:], d_kw[:])

            # q_aug fixed rows 64:68 = ones, ones, iota, iota
            for h in range(HLOC):
                nc.sync.dma_start(q_aug[h][64:68, :], d_qrows[:].bitcast(F32R))
            # v ones columns: one strided DMA over the whole v tile
            nc.sync.dma_start(
                vbig[:].rearrange("p (th d) -> p th d", d=65)[:, :, 64:65],
                d_ones64c[:].rearrange("p (th o) -> p th o", o=1),
            )

            iota4sb = persist.tile([HLOC, T], F32, tag="iota4", name="iota4")
            nc.sync.dma_start(iota4sb[:], d_iota4[:])
            eps8 = persist.tile([8, 1], F32, tag="eps8", name="eps8")
            nc.vector.memset(eps8[:], EPS_RMS)
            neghalf8 = persist.tile([8, 1], F32, tag="neghalf8", name="neghalf8")
            nc.vector.memset(neghalf8[:], -0.5)
            one4 = persist.tile([4, 1], F32, tag="one4", name="one4")
            nc.vector.memset(one4[:], 1.0)

            # ================= P2+P3: QKV, rms, gate, bias rows ============
            with ExitStack() as p2:
                qk_ps = p2.enter_context(
                    tc.tile_pool(name="qkps", bufs=3, space="PSUM")
                )
                rep_ps = p2.enter_context(
                    tc.tile_pool(name="repps", bufs=1, space="PSUM")
                )
                v_ps = p2.enter_context(
                    tc.tile_pool(name="vps", bufs=2, space="PSUM")
                )
                s8_ps = p2.enter_context(
                    tc.tile_pool(name="s8ps", bufs=1, space="PSUM")
                )
                st_ps = p2.enter_context(
                    tc.tile_pool(name="stps", bufs=1, space="PSUM")
                )
                sq_pool = p2.enter_context(tc.tile_pool(name="qsq", bufs=2))
                rep_sb = p2.enter_context(tc.tile_pool(name="repS", bufs=2))
                rsq_pool = p2.enter_context(tc.tile_pool(name="rsq", bufs=2))
                g_pool = p2.enter_context(tc.tile_pool(name="gate", bufs=1))
                st_pool = p2.enter_context(tc.tile_pool(name="stsb", bufs=1))

                def qk_batch(n, p, xt, sl):
                    """One head-pack p: q+k projection, batched rsqrt."""
                    ps_list = []
                    qsq_list = []
                    s4 = s8_ps.tile([4, NCH], F32, tag="s8", name=f"s4_{n}_{p}")
                    for loc in range(2):  # 0 = q, 1 = k
                        wsb = wk_sb if loc else wq_sb
                        ps = qk_ps.tile([128, NCH], F32, tag="qk", name="qk")
                        for cc in range(KC):
                            nc.tensor.matmul(
                                ps[:],
                                wsb[:, 256 * cc + 128 * p : 256 * cc + 128 * p + 128],
                                xt[:, 512 * cc : 512 * cc + 512],
                                start=(cc == 0),
                                stop=(cc == KC - 1),
                            )
                        ps_list.append(ps)
                        qsq = sq_pool.tile([128, NCH], F32R, tag="qsq", name="qsq")
                        nc.scalar.activation(qsq[:], ps[:], AF.Square)
                        qsq_list.append(qsq)
                    for loc, qsq in enumerate(qsq_list):
                        nc.tensor.matmul(
                            s4[:],
                            ssqw4[:, 4 * loc : 4 * loc + 4],
                            qsq[:],
                            start=(loc == 0),
                            stop=(loc == 1),
                            skip_group_check=True,
                        )
                    rsq_f = rsq_pool.tile([4, NCH], F32, tag="rsqf", name="rsqf")
                    nc.scalar.activation(rsq_f[:], s4[:], AF.Ln, bias=eps8[0:4, :])
                    rsq = rsq_pool.tile([4, NCH], F32R, tag="rsq", name="rsq")
                    nc.scalar.activation(
                        rsq[:], rsq_f[:], AF.Exp, scale=neghalf8[0:4, :]
                    )
                    return ps_list, rsq

                def rms_apply(p, ps_list, rsq, sl):
                    for loc, ps in enumerate(ps_list):
                        rep = rep_ps.tile([128, NCH], F32, tag="rep", name="rep")
                        nc.tensor.matmul(
                            rep[:],
                            selq[:, 128 * loc : 128 * loc + 128],
                            rsq[:],
                            start=True,
                            stop=True,
                        )
                        repS = rep_sb.tile([128, NCH], F32, tag="repS", name="repS")
                        nc.scalar.copy(repS[:], rep[:])
                        wcol = kw if loc else qw8
                        aug_set = k_aug if loc else q_aug
                        for s in range(2):
                            nc.vector.scalar_tensor_tensor(
                                aug_set[2 * p + s][0:64, sl],
                                ps[64 * s : 64 * s + 64, :],
                                wcol[64 * s : 64 * s + 64, :],
                                repS[64 * s : 64 * s + 64, :],
                                MUL,
                                MUL,
                            )

                def v_group(n, tl, xt):
                    t = 4 * n + tl
                    vps = v_ps.tile([128, NCH], F32, tag="vq", name="vps")
                    for cc in range(KC):
                        nc.tensor.matmul(
                            vps[:, 0:HD],
                            xt[:, 512 * cc + 128 * tl : 512 * cc + 128 * tl + 128],
                            wv_sb[:, 256 * cc : 256 * cc + 256],
                            start=(cc == 0),
                            stop=(cc == KC - 1),
                        )
                    dst = v_sb[t].rearrange("p (h d) -> p h d", h=HLOC)[:, :, 0:64]
                    nc.scalar.copy(
                        dst, vps[:, 0:HD].rearrange("p (h d) -> p h d", h=HLOC)
                    )

                for n in range(NT):
                    sl = slice(NCH * n, NCH * n + NCH)
                    if n + 2 < NT:
                        x_tiles[n + 2] = load_xn(n + 2)
                    xt = x_tiles.pop(n)

                    psA, rsqA = qk_batch(n, 0, xt, sl)
                    v_group(n, 0, xt)  # PE busy while ACT does batch-A rsqrt
                    v_group(n, 1, xt)
                    rms_apply(0, psA, rsqA, sl)
                    psB, rsqB = qk_batch(n, 1, xt, sl)
                    v_group(n, 2, xt)
                    v_group(n, 3, xt)
                    rms_apply(1, psB, rsqB, sl)


                    # --- P3: key gate + bias rows ---
                    g4 = s8_ps.tile([4, NCH], F32, tag="s8", name="g4")
                    for h in range(HLOC):
                        nc.tensor.matmul(
                            g4[:],
                            ucolblk[:, 4 * h : 4 * h + 4],
                            k_aug[h][0:64, sl],
                            start=(h == 0),
                            stop=(h == HLOC - 1),
                            skip_group_check=True,
                        )
                    gsc = g_pool.tile([4, NCH], F32, tag="gsc", name="gsc")
                    nc.scalar.activation(gsc[:], g4[:], AF.Exp)
                    gate4 = g_pool.tile([4, NCH], F32, tag="gate4", name="gate4")
                    nc.scalar.activation(gate4[:], gsc[:], AF.Ln, bias=one4[:])
                    a4f = g_pool.tile([4, NCH], F32, tag="a4f", name="a4f")
                    nc.vector.scalar_tensor_tensor(
                        a4f[:], gate4[:], omg[:], iota4sb[:, sl], MUL, MUL
                    )
                    w4f = g_pool.tile([4, NCH], F32, tag="w4f", name="w4f")
                    nc.vector.tensor_scalar_mul(w4f[:], gate4[:], negomg[:])
                    a_hi = g_pool.tile([4, NCH], F32R, tag="a_hi", name="a_hi")
                    nc.vector.tensor_copy(a_hi[:], a4f[:])
                    w_hi = g_pool.tile([4, NCH], F32R, tag="w_hi", name="w_hi")
                    nc.vector.tensor_copy(w_hi[:], w4f[:])
                    # lo-parts (exact f32 residuals) on the idle GPSIMD engine,
                    # in parallel with the DVE copies above
                    a_lo = g_pool.tile([4, NCH], F32R, tag="a_lo", name="a_lo")
                    nc.vector.scalar_tensor_tensor(
                        a_lo[:], a4f[:], 1.0, a_hi[:].bitcast(F32), MUL, SUB
                    )
                    w_lo = g_pool.tile([4, NCH], F32R, tag="w_lo", name="w_lo")
                    nc.vector.scalar_tensor_tensor(
                        w_lo[:], w4f[:], 1.0, w_hi[:].bitcast(F32), MUL, SUB
                    )
                    # stack head-major: st[4h+r] = S_r[h]
                    st16 = st_ps.tile([16, NCH], F32, tag="st16", name="st16")
                    for r, srcr in enumerate((a_hi, a_lo, w_hi, w_lo)):
                        nc.tensor.matmul(
                            st16[:],
                            scat[:, 16 * r : 16 * r + 16],
                            srcr[:],
                            start=(r == 0),
                            stop=(r == 3),
                            skip_group_check=True,
                        )
                    stsb = st_pool.tile([16, NCH], F32R, tag="stsb", name="stsb")
                    nc.vector.tensor_copy(stsb[:], st16[:])
                    for h in range(HLOC):
                        nc.sync.dma_start(
                            k_aug[h][64:68, sl], stsb[4 * h : 4 * h + 4, :]
                        )

            # ================= P4 + P5: attention & projection =============
            # Score tiles processed in PAIRS living in [128,1024] 2-bank PSUM
            # tiles; one Exp per pair. Diagonal pairs exp a few extra
            # (never-read) columns so the access pattern stays rectangular.
            with ExitStack() as p4:
                s_ps_pool = p4.enter_context(
                    tc.tile_pool(name="sps2", bufs=cfg["sps_bufs"], space="PSUM")
                )
                y_ps_pool = p4.enter_context(
                    tc.tile_pool(name="yps", bufs=2, space="PSUM")
                )
                o_ps_pool = p4.enter_context(
                    tc.tile_pool(name="ops", bufs=2, space="PSUM")
                )
                if cfg.get("rep_own"):
                    r_ps_pool = p4.enter_context(
                        tc.tile_pool(name="rps", bufs=1, space="PSUM")
                    )
                else:
                    r_ps_pool = o_ps_pool
                p_pool = p4.enter_context(tc.tile_pool(name="p", bufs=cfg["p_bufs"]))
                rcp_pool = p4.enter_context(tc.tile_pool(name="rcp", bufs=2))
                rep4_sb = p4.enter_context(tc.tile_pool(name="rep4", bufs=2))
                out_pool = p4.enter_context(tc.tile_pool(name="osb", bufs=2))

                def score_tile_mms(ci, h, dst, base, tj, smt_cap=None):
                    r = tj - 4 * ci
                    off = 0 if r < 0 else 128 * r
                    smt = min(off, 256)
                    if smt_cap is not None:
                        smt = min(smt, smt_cap)
                    nc.tensor.matmul(
                        dst[:, base + smt : base + NCH],
                        k_aug[h][:, 128 * tj : 128 * tj + 128],
                        q_aug[h][:, NCH * ci + smt : NCH * ci + NCH],
                        start=True,
                        stop=(r < 0),
                        skip_group_check=True,
                    )
                    if r >= 0:
                        # causal stair mask via constant matmul
                        nc.tensor.matmul(
                            dst[:, base + off : base + off + 128],
                            stairT[:],
                            ident[:],
                            start=False,
                            stop=True,
                            skip_group_check=True,
                        )
                    return off

                def pv_mm(ci, h, yps, psb, base, off, tj):
                    nc.tensor.matmul(
                        yps[:, off:NCH],
                        v_sb[tj][:, 65 * h : 65 * h + 65],
                        psb[:, base + off : base + NCH],
                        start=(tj == 0),
                        stop=(tj == 4 * ci + 3),
                        skip_group_check=True,
                    )

                def emit_score_pair(ci, h, yps, tj0, pending):
                    """Emit score mms + exp; PV matmuls are deferred by one
                    stage (pending list) so parked PVs never stall PE issue."""
                    if cfg["paired"]:
                        sps2 = s_ps_pool.tile(
                            [128, 2 * NCH], F32, tag="sps2", name="sps2"
                        )
                        r0 = tj0 - 4 * ci
                        cap = min(0 if r0 < 0 else 128 * r0, 256)
                        offs = [
                            score_tile_mms(ci, h, sps2, NCH * ti, tj0 + ti, cap)
                            for ti in range(2)
                        ]
                        # one exp for the pair, rectangular over both halves
                        # from min(offs) (extra cols never read)
                        eoff = offs[0]
                        psb = p_pool.tile(
                            [128, 2 * NCH], BF16, tag="p", name="p"
                        )
                        nc.scalar.activation(
                            psb[:]
                            .rearrange("p (t c) -> p t c", t=2)[:, :, eoff:NCH],
                            sps2[:]
                            .rearrange("p (t c) -> p t c", t=2)[:, :, eoff:NCH],
                            AF.Exp,
                        )
                        for ti in range(2):
                            pending.append(
                                (ci, h, yps, psb, NCH * ti, offs[ti], tj0 + ti)
                            )
                    else:
                        for ti in range(2):
                            tj = tj0 + ti
                            sps = s_ps_pool.tile(
                                [128, NCH], F32, tag="sps2", name="sps"
                            )
                            off = score_tile_mms(ci, h, sps, 0, tj)
                            psb = p_pool.tile(
                                [128, NCH], BF16, tag="p", name="p"
                            )
                            nc.scalar.activation(
                                psb[:, off:NCH], sps[:, off:NCH], AF.Exp
                            )
                            pending.append((ci, h, yps, psb, 0, off, tj))

                def flush_pv(pending, keep=0):
                    while len(pending) > keep:
                        pv_mm(*pending.pop(0))

                def emit_norm(ci, h, yps):
                    isl = slice(NCH * ci, NCH * ci + NCH)
                    p_pk, s_slot = divmod(h, 2)
                    # normalize: DVE reciprocal + PE broadcast (into the
                    # ops slot, which is idle until this chunk's P5)
                    rcp = rcp_pool.tile([1, NCH], F32R, tag="rcp", name="rcp")
                    with nc.allow_low_precision(reason="softmax denom bcast"):
                        nc.vector.reciprocal(rcp[:], yps[64:65, :])
                    repp = r_ps_pool.tile(
                        [128, NCH],
                        F32,
                        tag="rps" if cfg.get("rep_own") else "ops",
                        name="rep64",
                    )
                    nc.tensor.matmul(
                        repp[0:64, :], ones64[:], rcp[:], start=True, stop=True
                    )
                    repS = rep4_sb.tile([64, NCH], F32, tag="rep4", name="rep4")
                    nc.vector.tensor_copy(repS[:], repp[0:64, :])
                    nc.vector.tensor_tensor(
                        y_pack[p_pk][64 * s_slot : 64 * s_slot + 64, isl],
                        yps[0:64, :],
                        repS[:],
                        MUL,
                    )

                for ci in range(NT):
                    keep = cfg.get("pv_defer", 2)
                    if cfg["alternate"]:
                        # two heads in flight: alternate pair emission so one
                        # head's PE work hides the other's exp latency
                        for hp in range(2):
                            ha, hb = 2 * hp, 2 * hp + 1
                            ypsa = y_ps_pool.tile(
                                [65, NCH], F32, tag="yps", name="ypsa"
                            )
                            ypsb = y_ps_pool.tile(
                                [65, NCH], F32, tag="yps", name="ypsb"
                            )
                            pending = []
                            for tj0 in range(0, 4 * ci + 4, 2):
                                emit_score_pair(ci, ha, ypsa, tj0, pending)
                                flush_pv(pending, keep)
                                emit_score_pair(ci, hb, ypsb, tj0, pending)
                                flush_pv(pending, keep)
                            flush_pv(pending)
                            emit_norm(ci, ha, ypsa)
                            emit_norm(ci, hb, ypsb)
                    else:
                        for h in range(HLOC):
                            yps = y_ps_pool.tile(
                                [65, NCH], F32, tag="yps", name="yps"
                            )
                            pending = []
                            for tj0 in range(0, 4 * ci + 4, 2):
                                emit_score_pair(ci, h, yps, tj0, pending)
                                flush_pv(pending, keep)
                            flush_pv(pending)
                            emit_norm(ci, h, yps)
                    # P5: projection for the 4 t-tiles covered by this chunk
                    for tt in range(4 * ci, 4 * ci + 4):
                        osb = out_pool.tile([128, 1024], F32, tag="osb", name="osb")
                        for cn in range(2):
                            osl = slice(512 * cn, 512 * cn + 512)
                            ops = o_ps_pool.tile(
                                [128, NCH], F32, tag="ops", name="ops"
                            )
                            for p in range(2):
                                nc.tensor.matmul(
                                    ops[:],
                                    y_pack[p][:, 128 * tt : 128 * tt + 128],
                                    wproj_sb[:, 1024 * p + 512 * cn : 1024 * p + 512 * cn + 512],
                                    start=(p == 0),
                                    stop=(p == 1),
                                )
                            nc.vector.tensor_copy(osb[:, osl], ops[:])
                        nc.sync.dma_start(
                            d_out[128 * tt : 128 * tt + 128, :], osb[:]
                        )

    split_excess_waits(nc, max_waits=1)
    return nc


def _host_shards(inputs):
    x = np.asarray(inputs["x"], np.float32)
    Wq = np.asarray(inputs["Wq"], np.float32)
    Wk = np.asarray(inputs["Wk"], np.float32)
    Wv = np.asarray(inputs["Wv"], np.float32)
    Wproj = np.asarray(inputs["Wproj"], np.float32)
    q_rms_w = np.asarray(inputs["q_rms_w"], np.float32)
    k_rms_w = np.asarray(inputs["k_rms_w"], np.float32)
    omega = np.asarray(inputs["omega"], np.float32)
    u = np.asarray(inputs["u"], np.float32)

    import ml_dtypes

    slopes = np.asarray(_get_alibi_slopes(H), np.float32)
    omega_eff = np.log1p(np.exp(omega)) * slopes  # softplus(omega) * slopes
    u_n = u / np.maximum(np.linalg.norm(u, axis=-1, keepdims=True), U_L2_EPS)
    sqrt_d = math.sqrt(D)

    iota = np.arange(T, dtype=np.float32)[None, :]
    qrows = np.concatenate(
        [np.ones((2, T), np.float32), np.tile(iota, (2, 1))], axis=0
    )
    ones64c = np.ones((128, JT * HLOC), np.float32).astype(ml_dtypes.bfloat16)
    ones64 = np.ones((1, 64), np.float32)
    # selq [4, 256]: block loc: selq[2*loc + (m>=64), 128*loc + m] = 1
    selq = np.zeros((4, 256), np.float32)
    for loc in range(2):
        for m in range(128):
            selq[2 * loc + (m >= 64), 128 * loc + m] = 1.0
    jj = np.arange(128, dtype=np.float32)
    stair = np.where(jj[None, :] >= jj[:, None], 0.0, NEG_BIG).astype(np.float32)
    stairT = stair.T.astype(ml_dtypes.bfloat16)
    ident = np.eye(128, dtype=np.float32).astype(ml_dtypes.bfloat16)
    # ssqw4 [128, 8]: block loc (cols 4*loc..+4): col 4*loc + 2*loc + s <- 1/D
    # on rows 64s.. (s4 rows are 2*loc + s)
    ssqw4 = np.zeros((128, 8), np.float32)
    for loc in range(2):
        for s in range(2):
            ssqw4[64 * s : 64 * s + 64, 4 * loc + 2 * loc + s] = 1.0 / D

    # scat [4, 64]: block r: scat[h, 16r + 4h + r] = 1
    scat = np.zeros((4, 64), np.float32)
    for r in range(4):
        for h in range(4):
            scat[h, 16 * r + 4 * h + r] = 1.0
    qw8 = np.tile(q_rms_w / 8.0, 2)[:, None].astype(np.float32)
    kw = np.tile(k_rms_w, 2)[:, None].astype(np.float32)

    in_maps = []
    for core in range(8):
        b, g = divmod(core, HLOC)
        hs = slice(HLOC * g, HLOC * g + HLOC)
        cs = slice(HD * g, HD * g + HD)
        # ucolblk [64, 16]: col 4h+j = u_n[head h]/sqrt(D) if j==h else 0
        ucolblk = np.zeros((D, 16), np.float32)
        for h in range(HLOC):
            ucolblk[:, 4 * h + h] = u_n[HLOC * g + h] / sqrt_d
        in_maps.append(
            {
                "xT": np.ascontiguousarray(x[b].T),
                "wq": np.ascontiguousarray(Wq[:, cs]),
                "wk": np.ascontiguousarray(Wk[:, cs]),
                "wv": np.ascontiguousarray(Wv[:, cs]),
                "wproj": np.ascontiguousarray(Wproj[cs, :]),
                "ucolblk": ucolblk,
                "omg": np.ascontiguousarray(omega_eff[hs][:, None]),
                "negomg": np.ascontiguousarray(-omega_eff[hs][:, None]),
                "iota4": np.tile(iota, (HLOC, 1)),
                "qrows": qrows,
                "ones64c": ones64c,
                "stairT": stairT,
                "ident": ident,
                "scat": scat,
                "selq": selq,
                "ones64": ones64,
                "ssqw4": ssqw4,
                "qw8": qw8,
                "kw": kw,
            }
        )
    return in_maps


def kernel(**inputs):
    from concourse.bass_utils import run_bass_kernel_spmd

    if "nc" not in _cache:
        _cache["nc"] = _build_program()
    nc = _cache["nc"]

    in_maps = _host_shards(inputs)
    res = run_bass_kernel_spmd(nc, in_maps, core_ids=list(range(8)))
    out = np.zeros((B, T, C), np.float32)
    for core in range(8):
        b = core // HLOC
        out[b] += res.results[core]["out"]
    return out


# revision 26
# speedup vs baseline: 1.8062x; 1.0564x over previous
"""Causal self-attention (RMSNorm QK, key-gated ALiBi bias) on 8 TRN2 cores.

Sharding: data-parallel over batch (2) x tensor-parallel over heads (4 groups
of 4 heads) = 8 cores. Each core computes a partial c_proj output for its
batch; the host sums the 4 head-group partials per batch.

Device kernel v2 (restructured from the 347us baseline):
  - Inputs stream in as a few large rearranged DMAs; QKV matmuls start on
    the first T-chunk while later chunks load (kills the DMA-only lead-in).
  - RMS rsqrt batched: sum-of-squares rows for all 4 (pack, q/k) combos are
    stacked into one [8,512] PSUM tile by matmul, one Ln + one Exp per chunk.
  - rsqrt/denominator broadcasts over 64 partitions via PE selector matmuls
    (no DRAM roundtrips).
  - Key-gate softplus batched: gate logits for 4 heads stacked by matmul
    accumulation, one Exp + one Ln per chunk.
  - Bias rows (a_hi/a_lo/w_hi/w_lo) stacked head-major by matmul, one DVE
    copy + 4 DMAs per chunk.
  - Softmax denominator reciprocal on DVE (vector.reciprocal).
  - Causal stair mask folded into the score matmul accumulation group as a
    constant bf16 matmul (stairT^T @ I).
"""

import sys

if "/opt/trn_rl_repo" not in sys.path:
    sys.path.insert(0, "/opt/trn_rl_repo")

import math

import numpy as np

B, T, C = 2, 2048, 1024
H, D = 16, 64
HLOC = 4           # heads per core
HD = HLOC * D      # 256
NCH = 512          # T-chunk width
NT = T // NCH      # 4 chunks
JT = T // 128      # 16 j-tiles
KC = C // 128      # 8 contraction chunks
EPS_RMS = 1e-5
U_L2_EPS = 1e-6
NEG_BIG = -1.0e30

_cache = {}

# P4 emission config (sweepable)
CFG = {
    "paired": True,
    "alternate": True,
    "sps_bufs": 2,
    "p_bufs": 6,
    "rep_own": False,
    "pv_defer": 6,
}


def _get_alibi_slopes(n_heads):
    def pow2(n):
        start = 2 ** (-(2 ** (-(math.log2(n) - 3))))
        return [start * start**i for i in range(n)]

    if math.log2(n_heads).is_integer():
        return pow2(n_heads)
    c = 2 ** math.floor(math.log2(n_heads))
    s = pow2(c)
    extra = _get_alibi_slopes(2 * c)
    return s + extra[0::2][: n_heads - c]


def _build_program(cfg=None):
    cfg = dict(CFG if cfg is None else cfg)
    import concourse.bass as bass
    import concourse.mybir as mybir
    import concourse.tile as tile
    from concourse.alu_op_type import AluOpType
    from concourse.vector_clock import ScopedClock

    F32 = mybir.dt.float32
    F32R = mybir.dt.float32r
    BF16 = mybir.dt.bfloat16
    AF = mybir.ActivationFunctionType
    MUL = AluOpType.mult
    SUB = AluOpType.subtract

    class PatchedTileContext(tile.TileContext):
        """Tail drain split into nops carrying <=2 sem waits each (this
        walrus build rejects CTRL instructions with more)."""

        def _drain_and_barrier(self, tick_clock, wait_clock):
            nc = self.nc
            probe = nc.sync.nop(nofuse=True)
            wait_clock.add_sem_waits(
                probe.ins, ScopedClock({None: tick_clock.global_clock})
            )
            si = probe.ins.sync_info
            waits = list(si.on_wait or []) if si is not None else []
            if len(waits) > 2:
                si.on_wait = waits[:2]
                rest = waits[2:]
                for i in range(0, len(rest), 2):
                    extra = nc.sync.nop(nofuse=True)
                    esi = extra.ins.sync_info
                    chunk = rest[i : i + 2]
                    if esi is None:
                        extra.ins.sync_info = mybir.SyncInfo(
                            on_wait=chunk, on_update=[]
                        )
                    else:
                        esi.on_wait = (esi.on_wait or []) + chunk
            nc.sync.drain()
            nc.all_engine_barrier()
            assert self.sems is not None
            popped = nc._tile_sem_poison_stack.pop()
            assert popped is self._sem_poison
            nc.clear_and_free_semaphores(list(self.sems.allocated().values()))
            nc.all_engine_barrier()

    def split_excess_waits(nc, max_waits=1):
        for f in nc.m.functions:
            for blk in f.blocks:
                new_insts = []
                for inst in blk.instructions:
                    si = inst.sync_info
                    if si is not None and si.on_wait and len(si.on_wait) > max_waits:
                        waits = list(si.on_wait)
                        si.on_wait = waits[-max_waits:]
                        rest = waits[:-max_waits]
                        for i in range(0, len(rest), max_waits):
                            nop = mybir.InstNoOp(
                                name=f"I-waitsplit-{nc.next_id()}",
                                ins=[],
                                outs=[],
                                engine=inst.engine,
                                sync_info=mybir.SyncInfo(
                                    on_wait=rest[i : i + max_waits], on_update=[]
                                ),
                            )
                            nc.register_instruction(nop)
                            new_insts.append(nop)
                    new_insts.append(inst)
                blk.instructions = new_insts

    nc = bass.Bass(trn_type="TRN2", num_devices=8, debug=False)

    # ---- DRAM I/O (per-core shards supplied by the host) ----
    d_xT = nc.dram_tensor("xT", [C, T], BF16, kind="ExternalInput")
    d_wq = nc.dram_tensor("wq", [C, HD], BF16, kind="ExternalInput")
    d_wk = nc.dram_tensor("wk", [C, HD], BF16, kind="ExternalInput")
    d_wv = nc.dram_tensor("wv", [C, HD], BF16, kind="ExternalInput")
    d_wproj = nc.dram_tensor("wproj", [HD, C], F32, kind="ExternalInput")
    d_ucolblk = nc.dram_tensor("ucolblk", [D, 16], F32, kind="ExternalInput")
    d_omg = nc.dram_tensor("omg", [HLOC, 1], F32, kind="ExternalInput")
    d_negomg = nc.dram_tensor("negomg", [HLOC, 1], F32, kind="ExternalInput")
    d_iota4 = nc.dram_tensor("iota4", [HLOC, T], F32, kind="ExternalInput")
    d_qrows = nc.dram_tensor("qrows", [4, T], F32, kind="ExternalInput")
    d_ones64c = nc.dram_tensor("ones64c", [128, JT * HLOC], BF16, kind="ExternalInput")
    d_stairT = nc.dram_tensor("stairT", [128, 128], BF16, kind="ExternalInput")
    d_ident = nc.dram_tensor("ident", [128, 128], BF16, kind="ExternalInput")
    d_scat = nc.dram_tensor("scat", [4, 64], F32, kind="ExternalInput")
    d_selq = nc.dram_tensor("selq", [4, 256], F32, kind="ExternalInput")
    d_ones64 = nc.dram_tensor("ones64", [1, 64], F32, kind="ExternalInput")
    d_ssqw4 = nc.dram_tensor("ssqw4", [128, 8], F32, kind="ExternalInput")
    d_qw8 = nc.dram_tensor("qw8", [128, 1], F32, kind="ExternalInput")
    d_kw = nc.dram_tensor("kw", [128, 1], F32, kind="ExternalInput")
    d_out = nc.dram_tensor("out", [T, C], F32, kind="ExternalOutput")

    with PatchedTileContext(nc) as tc:
        from contextlib import ExitStack

        with ExitStack() as top:
            persist = top.enter_context(tc.tile_pool(name="persist", bufs=1))

            # ---- persistent SBUF tensors ----
            q_aug = [persist.tile([68, T], F32R, tag=f"qaug{h}", name=f"qaug{h}") for h in range(HLOC)]
            k_aug = [persist.tile([68, T], F32R, tag=f"kaug{h}", name=f"kaug{h}") for h in range(HLOC)]
            vbig = persist.tile([128, JT * HLOC * 65], BF16, tag="vbig", name="vbig")
            v_sb = [vbig[:, 260 * t : 260 * t + 260] for t in range(JT)]
            y_pack = [
                persist.tile([128, T], F32R, tag=f"ypk{p}", name=f"ypk{p}")
                for p in range(2)
            ]

            # ---- weights: one rearranged DMA each ----
            wq_sb = persist.tile([128, 2048], BF16, tag="wq", name="wq")
            wk_sb = persist.tile([128, 2048], BF16, tag="wk", name="wk")
            wv_sb = persist.tile([128, 2048], BF16, tag="wv", name="wv")
            wproj_sb = persist.tile([128, 2048], F32R, tag="wproj", name="wproj")
            def load_w(wsb, dten, half=None):
                halves = range(2) if half is None else [half]
                for hf in halves:
                    nc.sync.dma_start(
                        wsb[:, 1024 * hf : 1024 * hf + 1024].rearrange(
                            "p (c j) -> p c j", c=KC // 2
                        ),
                        dten[512 * hf : 512 * hf + 512, :].rearrange(
                            "(c p) j -> p c j", p=128
                        ),
                    )

            # ---- x chunks: 2 DMAs per T-chunk (4 contraction chunks each) ----
            xpool = top.enter_context(tc.tile_pool(name="xT", bufs=1))

            def load_xn_half(xt, n, hf):
                sl = slice(NCH * n, NCH * n + NCH)
                src = d_xT[512 * hf : 512 * hf + 512, sl].rearrange(
                    "(c p) t -> p c t", p=128
                )
                dst = xt[:, 2048 * hf : 2048 * hf + 2048].rearrange(
                    "p (c t) -> p c t", c=4
                )
                nc.sync.dma_start(dst, src)

            def load_xn(n):
                xt = xpool.tile([128, 4096], BF16, tag=f"x{n % 2}", name=f"x{n}")
                load_xn_half(xt, n, 0)
                load_xn_half(xt, n, 1)
                return xt

            # interleave the first x chunk with the q/k weights so the first
            # projection matmuls can start as early as possible; x1 right
            # after wk so chunk n=1 is never starved behind const DMAs
            load_w(wq_sb, d_wq, half=0)
            x0 = xpool.tile([128, 4096], BF16, tag="x0", name="x_0")
            load_xn_half(x0, 0, 0)
            load_w(wq_sb, d_wq, half=1)
            load_xn_half(x0, 0, 1)
            x_tiles = {0: x0}
            load_w(wk_sb, d_wk)
            x_tiles[1] = load_xn(1)
            load_w(wv_sb, d_wv)
            nc.sync.dma_start(
                wproj_sb[:].rearrange("p (g j) -> p g j", g=2),
                d_wproj[:].bitcast(F32R).rearrange("(g p) j -> p g j", p=128),
            )

            stairT = persist.tile([128, 128], BF16, tag="stairT", name="stairT")
            nc.sync.dma_start(stairT[:], d_stairT[:])
            ident = persist.tile([128, 128], BF16, tag="ident", name="ident")
            nc.sync.dma_start(ident[:], d_ident[:])
            scat = persist.tile([4, 64], F32R, tag="scat", name="scat")
            nc.sync.dma_start(scat[:], d_scat[:].bitcast(F32R))
            selq = persist.tile([4, 256], F32R, tag="selq", name="selq")
            nc.sync.dma_start(selq[:], d_selq[:].bitcast(F32R))
            ones64 = persist.tile([1, 64], F32R, tag="ones64", name="ones64")
            nc.sync.dma_start(ones64[:], d_ones64[:].bitcast(F32R))
            ssqw4 = persist.tile([128, 8], F32R, tag="ssqw4", name="ssqw4")
            nc.sync.dma_start(ssqw4[:], d_ssqw4[:].bitcast(F32R))
            ucolblk = persist.tile([D, 16], F32R, tag="ucolblk", name="ucolblk")
            nc.sync.dma_start(ucolblk[:], d_ucolblk[:].bitcast(F32R))
            omg = persist.tile([HLOC, 1], F32, tag="omg", name="omg")
            nc.sync.dma_start(omg[:], d_omg[:])
            negomg = persist.tile([HLOC, 1], F32, tag="negomg", name="negomg")
            nc.sync.dma_start(negomg[:], d_negomg[:])
            qw8 = persist.tile([128, 1], F32, tag="qw8", name="qw8")
            nc.sync.dma_start(qw8[:], d_qw8[:])
            kw = persist.tile([128, 1], F32, tag="kw", name="kw")
            nc.sync.dma_start(kw[:], d_kw[:])

            # q_aug fixed rows 64:68 = ones, ones, iota, iota
            for h in range(HLOC):
                nc.sync.dma_start(q_aug[h][64:68, :], d_qrows[:].bitcast(F32R))
            # v ones columns: one strided DMA over the whole v tile
            nc.sync.dma_start(
                vbig[:].rearrange("p (th d) -> p th d", d=65)[:, :, 64:65],
                d_ones64c[:].rearrange("p (th o) -> p th o", o=1),
            )

            iota4sb = persist.tile([HLOC, T], F32, tag="iota4", name="iota4")
            nc.sync.dma_start(iota4sb[:], d_iota4[:])
            eps8 = persist.tile([8, 1], F32, tag="eps8", name="eps8")
            nc.vector.memset(eps8[:], EPS_RMS)
            neghalf8 = persist.tile([8, 1], F32, tag="neghalf8", name="neghalf8")
            nc.vector.memset(neghalf8[:], -0.5)
            one4 = persist.tile([4, 1], F32, tag="one4", name="one4")
            nc.vector.memset(one4[:], 1.0)

            # ================= P2+P3: QKV, rms, gate, bias rows ============
            with ExitStack() as p2:
                qk_ps = p2.enter_context(
                    tc.tile_pool(name="qkps", bufs=3, space="PSUM")
                )
                rep_ps = p2.enter_context(
                    tc.tile_pool(name="repps", bufs=1, space="PSUM")
                )
                v_ps = p2.enter_context(
                    tc.tile_pool(name="vps", bufs=2, space="PSUM")
                )
                s8_ps = p2.enter_context(
                    tc.tile_pool(name="s8ps", bufs=1, space="PSUM")
                )
                st_ps = p2.enter_context(
                    tc.tile_pool(name="stps", bufs=1, space="PSUM")
                )
                sq_pool = p2.enter_context(tc.tile_pool(name="qsq", bufs=2))
                rep_sb = p2.enter_context(tc.tile_pool(name="repS", bufs=2))
                rsq_pool = p2.enter_context(tc.tile_pool(name="rsq", bufs=2))
                g_pool = p2.enter_context(tc.tile_pool(name="gate", bufs=1))
                st_pool = p2.enter_context(tc.tile_pool(name="stsb", bufs=1))

                def qk_batch(n, p, xt, sl):
                    """One head-pack p: q+k projection, batched rsqrt."""
                    ps_list = []
                    qsq_list = []
                    s4 = s8_ps.tile([4, NCH], F32, tag="s8", name=f"s4_{n}_{p}")
                    for loc in range(2):  # 0 = q, 1 = k
                        wsb = wk_sb if loc else wq_sb
                        ps = qk_ps.tile([128, NCH], F32, tag="qk", name="qk")
                        for cc in range(KC):
                            nc.tensor.matmul(
                                ps[:],
                                wsb[:, 256 * cc + 128 * p : 256 * cc + 128 * p + 128],
                                xt[:, 512 * cc : 512 * cc + 512],
                                start=(cc == 0),
                                stop=(cc == KC - 1),
                            )
                        ps_list.append(ps)
                        qsq = sq_pool.tile([128, NCH], F32R, tag="qsq", name="qsq")
                        nc.scalar.activation(qsq[:], ps[:], AF.Square)
                        qsq_list.append(qsq)
                    for loc, qsq in enumerate(qsq_list):
                        nc.tensor.matmul(
                            s4[:],
                            ssqw4[:, 4 * loc : 4 * loc + 4],
                            qsq[:],
                            start=(loc == 0),
                            stop=(loc == 1),
                            skip_group_check=True,
                        )
                    rsq_f = rsq_pool.tile([4, NCH], F32, tag="rsqf", name="rsqf")
                    nc.scalar.activation(rsq_f[:], s4[:], AF.Ln, bias=eps8[0:4, :])
                    rsq = rsq_pool.tile([4, NCH], F32R, tag="rsq", name="rsq")
                    nc.scalar.activation(
                        rsq[:], rsq_f[:], AF.Exp, scale=neghalf8[0:4, :]
                    )
                    return ps_list, rsq

                def rms_apply(p, ps_list, rsq, sl):
                    for loc, ps in enumerate(ps_list):
                        rep = rep_ps.tile([128, NCH], F32, tag="rep", name="rep")
                        nc.tensor.matmul(
                            rep[:],
                            selq[:, 128 * loc : 128 * loc + 128],
                            rsq[:],
                            start=True,
                            stop=True,
                        )
                        repS = rep_sb.tile([128, NCH], F32, tag="repS", name="repS")
                        nc.scalar.copy(repS[:], rep[:])
                        wcol = kw if loc else qw8
                        aug_set = k_aug if loc else q_aug
                        for s in range(2):
                            nc.vector.scalar_tensor_tensor(
                                aug_set[2 * p + s][0:64, sl],
                                ps[64 * s : 64 * s + 64, :],
                                wcol[64 * s : 64 * s + 64, :],
                                repS[64 * s : 64 * s + 64, :],
                                MUL,
                                MUL,
                            )

                def v_group(n, tl, xt):
                    t = 4 * n + tl
                    vps = v_ps.tile([128, NCH], F32, tag="vq", name="vps")
                    for cc in range(KC):
                        nc.tensor.matmul(
                            vps[:, 0:HD],
                            xt[:, 512 * cc + 128 * tl : 512 * cc + 128 * tl + 128],
                            wv_sb[:, 256 * cc : 256 * cc + 256],
                            start=(cc == 0),
                            stop=(cc == KC - 1),
                        )
                    dst = v_sb[t].rearrange("p (h d) -> p h d", h=HLOC)[:, :, 0:64]
                    nc.scalar.copy(
                        dst, vps[:, 0:HD].rearrange("p (h d) -> p h d", h=HLOC)
                    )

                for n in range(NT):
                    sl = slice(NCH * n, NCH * n + NCH)
                    if n + 2 < NT:
                        x_tiles[n + 2] = load_xn(n + 2)
                    xt = x_tiles.pop(n)

                    psA, rsqA = qk_batch(n, 0, xt, sl)
                    v_group(n, 0, xt)  # PE busy while ACT does batch-A rsqrt
                    v_group(n, 1, xt)
                    rms_apply(0, psA, rsqA, sl)
                    psB, rsqB = qk_batch(n, 1, xt, sl)
                    v_group(n, 2, xt)
                    v_group(n, 3, xt)
                    rms_apply(1, psB, rsqB, sl)


                    # --- P3: key gate + bias rows ---
                    g4 = s8_ps.tile([4, NCH], F32, tag="s8", name="g4")
                    for h in range(HLOC):
                        nc.tensor.matmul(
                            g4[:],
                            ucolblk[:, 4 * h : 4 * h + 4],
                            k_aug[h][0:64, sl],
                            start=(h == 0),
                            stop=(h == HLOC - 1),
                            skip_group_check=True,
                        )
                    gsc = g_pool.tile([4, NCH], F32, tag="gsc", name="gsc")
                    nc.scalar.activation(gsc[:], g4[:], AF.Exp)
                    gate4 = g_pool.tile([4, NCH], F32, tag="gate4", name="gate4")
                    nc.scalar.activation(gate4[:], gsc[:], AF.Ln, bias=one4[:])
                    a4f = g_pool.tile([4, NCH], F32, tag="a4f", name="a4f")
                    nc.vector.scalar_tensor_tensor(
                        a4f[:], gate4[:], omg[:], iota4sb[:, sl], MUL, MUL
                    )
                    w4f = g_pool.tile([4, NCH], F32, tag="w4f", name="w4f")
                    nc.vector.tensor_scalar_mul(w4f[:], gate4[:], negomg[:])
                    a_hi = g_pool.tile([4, NCH], F32R, tag="a_hi", name="a_hi")
                    nc.vector.tensor_copy(a_hi[:], a4f[:])
                    w_hi = g_pool.tile([4, NCH], F32R, tag="w_hi", name="w_hi")
                    nc.vector.tensor_copy(w_hi[:], w4f[:])
                    # lo-parts (exact f32 residuals) on the idle GPSIMD engine,
                    # in parallel with the DVE copies above
                    a_lo = g_pool.tile([4, NCH], F32R, tag="a_lo", name="a_lo")
                    nc.vector.scalar_tensor_tensor(
                        a_lo[:], a4f[:], 1.0, a_hi[:].bitcast(F32), MUL, SUB
                    )
                    w_lo = g_pool.tile([4, NCH], F32R, tag="w_lo", name="w_lo")
                    nc.vector.scalar_tensor_tensor(
                        w_lo[:], w4f[:], 1.0, w_hi[:].bitcast(F32), MUL, SUB
                    )
                    # stack head-major: st[4h+r] = S_r[h]
                    st16 = st_ps.tile([16, NCH], F32, tag="st16", name="st16")
                    for r, srcr in enumerate((a_hi, a_lo, w_hi, w_lo)):
                        nc.tensor.matmul(
                            st16[:],
                            scat[:, 16 * r : 16 * r + 16],
                            srcr[:],
                            start=(r == 0),
                            stop=(r == 3),
                            skip_group_check=True,
                        )
                    stsb = st_pool.tile([16, NCH], F32R, tag="stsb", name="stsb")
                    nc.vector.tensor_copy(stsb[:], st16[:])
                    for h in range(HLOC):
                        nc.sync.dma_start(
                            k_aug[h][64:68, sl], stsb[4 * h : 4 * h + 4, :]
                        )

            # ================= P4 + P5: attention & projection =============
            # Score tiles processed in PAIRS living in [128,1024] 2-bank PSUM
            # tiles; one Exp per pair. Diagonal pairs exp a few extra
            # (never-read) columns so the access pattern stays rectangular.
            with ExitStack() as p4:
                s_ps_pool = p4.enter_context(
                    tc.tile_pool(name="sps2", bufs=cfg["sps_bufs"], space="PSUM")
                )
                y_ps_pool = p4.enter_context(
                    tc.tile_pool(name="yps", bufs=2, space="PSUM")
                )
                o_ps_pool = p4.enter_context(
                    tc.tile_pool(name="ops", bufs=2, space="PSUM")
                )
                if cfg.get("rep_own"):
                    r_ps_pool = p4.enter_context(
                        tc.tile_pool(name="rps", bufs=1, space="PSUM")
                    )
                else:
                    r_ps_pool = o_ps_pool
                p_pool = p4.enter_context(tc.tile_pool(name="p", bufs=cfg["p_bufs"]))
                rcp_pool = p4.enter_context(tc.tile_pool(name="rcp", bufs=2))
                rep4_sb = p4.enter_context(tc.tile_pool(name="rep4", bufs=2))
                out_pool = p4.enter_context(tc.tile_pool(name="osb", bufs=2))

                def score_tile_mms(ci, h, dst, base, tj, smt_cap=None):
                    r = tj - 4 * ci
                    off = 0 if r < 0 else 128 * r
                    smt = min(off, 256)
                    if smt_cap is not None:
                        smt = min(smt, smt_cap)
                    nc.tensor.matmul(
                        dst[:, base + smt : base + NCH],
                        k_aug[h][:, 128 * tj : 128 * tj + 128],
                        q_aug[h][:, NCH * ci + smt : NCH * ci + NCH],
                        start=True,
                        stop=(r < 0),
                        skip_group_check=True,
                    )
                    if r >= 0:
                        # causal stair mask via constant matmul
                        nc.tensor.matmul(
                            dst[:, base + off : base + off + 128],
                            stairT[:],
                            ident[:],
                            start=False,
                            stop=True,
                            skip_group_check=True,
                        )
                    return off

                def pv_mm(ci, h, yps, psb, base, off, tj):
                    nc.tensor.matmul(
                        yps[:, off:NCH],
                        v_sb[tj][:, 65 * h : 65 * h + 65],
                        psb[:, base + off : base + NCH],
                        start=(tj == 0),
                        stop=(tj == 4 * ci + 3),
                        skip_group_check=True,
                    )

                def emit_score_pair(ci, h, yps, tj0, pending):
                    """Emit score mms + exp; PV matmuls are deferred by one
                    stage (pending list) so parked PVs never stall PE issue."""
                    if cfg["paired"]:
                        sps2 = s_ps_pool.tile(
                            [128, 2 * NCH], F32, tag="sps2", name="sps2"
                        )
                        r0 = tj0 - 4 * ci
                        cap = min(0 if r0 < 0 else 128 * r0, 256)
                        offs = [
                            score_tile_mms(ci, h, sps2, NCH * ti, tj0 + ti, cap)
                            for ti in range(2)
                        ]
                        # one exp for the pair, rectangular over both halves
                        # from min(offs) (extra cols never read)
                        eoff = offs[0]
                        psb = p_pool.tile(
                            [128, 2 * NCH], BF16, tag="p", name="p"
                        )
                        nc.scalar.activation(
                            psb[:]
                            .rearrange("p (t c) -> p t c", t=2)[:, :, eoff:NCH],
                            sps2[:]
                            .rearrange("p (t c) -> p t c", t=2)[:, :, eoff:NCH],
                            AF.Exp,
                        )
                        for ti in range(2):
                            pending.append(
                                (ci, h, yps, psb, NCH * ti, offs[ti], tj0 + ti)
                            )
                    else:
                        for ti in range(2):
                            tj = tj0 + ti
                            sps = s_ps_pool.tile(
                                [128, NCH], F32, tag="sps2", name="sps"
                            )
                            off = score_tile_mms(ci, h, sps, 0, tj)
                            psb = p_pool.tile(
                                [128, NCH], BF16, tag="p", name="p"
                            )
                            nc.scalar.activation(
                                psb[:, off:NCH], sps[:, off:NCH], AF.Exp
                            )
                            pending.append((ci, h, yps, psb, 0, off, tj))

                def flush_pv(pending, keep=0):
                    while len(pending) > keep:
                        pv_mm(*pending.pop(0))

                def emit_norm(ci, h, yps):
                    isl = slice(NCH * ci, NCH * ci + NCH)
                    p_pk, s_slot = divmod(h, 2)
                    # normalize: DVE reciprocal + PE broadcast (into the
                    # ops slot, which is idle until this chunk's P5)
                    rcp = rcp_pool.tile([1, NCH], F32R, tag="rcp", name="rcp")
                    with nc.allow_low_precision(reason="softmax denom bcast"):
                        nc.vector.reciprocal(rcp[:], yps[64:65, :])
                    repp = r_ps_pool.tile(
                        [128, NCH],
                        F32,
                        tag="rps" if cfg.get("rep_own") else "ops",
                        name="rep64",
                    )
                    nc.tensor.matmul(
                        repp[0:64, :], ones64[:], rcp[:], start=True, stop=True
                    )
                    repS = rep4_sb.tile([64, NCH], F32, tag="rep4", name="rep4")
                    nc.vector.tensor_copy(repS[:], repp[0:64, :])
                    nc.vector.tensor_tensor(
                        y_pack[p_pk][64 * s_slot : 64 * s_slot + 64, isl],
                        yps[0:64, :],
                        repS[:],
                        MUL,
                    )

                for ci in range(NT):
                    keep = cfg.get("pv_defer", 2)
                    if cfg["alternate"]:
                        # two heads in flight: alternate pair emission so one
                        # head's PE work hides the other's exp latency
                        for hp in range(2):
                            ha, hb = 2 * hp, 2 * hp + 1
                            ypsa = y_ps_pool.tile(
                                [65, NCH], F32, tag="yps", name="ypsa"
                            )
                            ypsb = y_ps_pool.tile(
                                [65, NCH], F32, tag="yps", name="ypsb"
                            )
                            pending = []
                            for tj0 in range(0, 4 * ci + 4, 2):
                                emit_score_pair(ci, ha, ypsa, tj0, pending)
                                flush_pv(pending, keep)
                                emit_score_pair(ci, hb, ypsb, tj0, pending)
                                flush_pv(pending, keep)
                            flush_pv(pending)
                            emit_norm(ci, ha, ypsa)
                            emit_norm(ci, hb, ypsb)
                    else:
                        for h in range(HLOC):
                            yps = y_ps_pool.tile(
                                [65, NCH], F32, tag="yps", name="yps"
                            )
                            pending = []
                            for tj0 in range(0, 4 * ci + 4, 2):
                                emit_score_pair(ci, h, yps, tj0, pending)
                                flush_pv(pending, keep)
                            flush_pv(pending)
                            emit_norm(ci, h, yps)
                    # P5: projection for the 4 t-tiles covered by this chunk
                    for tt in range(4 * ci, 4 * ci + 4):
                        osb = out_pool.tile([128, 1024], F32, tag="osb", name="osb")
                        for cn in range(2):
                            osl = slice(512 * cn, 512 * cn + 512)
                            ops = o_ps_pool.tile(
                                [128, NCH], F32, tag="ops", name="ops"
                            )
                            for p in range(2):
                                nc.tensor.matmul(
                                    ops[:],
                                    y_pack[p][:, 128 * tt : 128 * tt + 128],
                                    wproj_sb[:, 1024 * p + 512 * cn : 1024 * p + 512 * cn + 512],
                                    start=(p == 0),
                                    stop=(p == 1),
                                )
                            nc.vector.tensor_copy(osb[:, osl], ops[:])
                        nc.sync.dma_start(
                            d_out[128 * tt : 128 * tt + 128, :], osb[:]
                        )

    split_excess_waits(nc, max_waits=1)
    return nc


def _host_shards(inputs):
    x = np.asarray(inputs["x"], np.float32)
    Wq = np.asarray(inputs["Wq"], np.float32)
    Wk = np.asarray(inputs["Wk"], np.float32)
    Wv = np.asarray(inputs["Wv"], np.float32)
    Wproj = np.asarray(inputs["Wproj"], np.float32)
    q_rms_w = np.asarray(inputs["q_rms_w"], np.float32)
    k_rms_w = np.asarray(inputs["k_rms_w"], np.float32)
    omega = np.asarray(inputs["omega"], np.float32)
    u = np.asarray(inputs["u"], np.float32)

    import ml_dtypes

    slopes = np.asarray(_get_alibi_slopes(H), np.float32)
    omega_eff = np.log1p(np.exp(omega)) * slopes  # softplus(omega) * slopes
    u_n = u / np.maximum(np.linalg.norm(u, axis=-1, keepdims=True), U_L2_EPS)
    sqrt_d = math.sqrt(D)

    iota = np.arange(T, dtype=np.float32)[None, :]
    qrows = np.concatenate(
        [np.ones((2, T), np.float32), np.tile(iota, (2, 1))], axis=0
    )
    ones64c = np.ones((128, JT * HLOC), np.float32).astype(ml_dtypes.bfloat16)
    ones64 = np.ones((1, 64), np.float32)
    # selq [4, 256]: block loc: selq[2*loc + (m>=64), 128*loc + m] = 1
    selq = np.zeros((4, 256), np.float32)
    for loc in range(2):
        for m in range(128):
            selq[2 * loc + (m >= 64), 128 * loc + m] = 1.0
    jj = np.arange(128, dtype=np.float32)
    stair = np.where(jj[None, :] >= jj[:, None], 0.0, NEG_BIG).astype(np.float32)
    stairT = stair.T.astype(ml_dtypes.bfloat16)
    ident = np.eye(128, dtype=np.float32).astype(ml_dtypes.bfloat16)
    # ssqw4 [128, 8]: block loc (cols 4*loc..+4): col 4*loc + 2*loc + s <- 1/D
    # on rows 64s.. (s4 rows are 2*loc + s)
    ssqw4 = np.zeros((128, 8), np.float32)
    for loc in range(2):
        for s in range(2):
            ssqw4[64 * s : 64 * s + 64, 4 * loc + 2 * loc + s] = 1.0 / D

    # scat [4, 64]: block r: scat[h, 16r + 4h + r] = 1
    scat = np.zeros((4, 64), np.float32)
    for r in range(4):
        for h in range(4):
            scat[h, 16 * r + 4 * h + r] = 1.0
    qw8 = np.tile(q_rms_w / 8.0, 2)[:, None].astype(np.float32)
    kw = np.tile(k_rms_w, 2)[:, None].astype(np.float32)

    in_maps = []
    for core in range(8):
        b, g = divmod(core, HLOC)
        hs = slice(HLOC * g, HLOC * g + HLOC)
        cs = slice(HD * g, HD * g + HD)
        # ucolblk [64, 16]: col 4h+j = u_n[head h]/sqrt(D) if j==h else 0
        ucolblk = np.zeros((D, 16), np.float32)
        for h in range(HLOC):
            ucolblk[:, 4 * h + h] = u_n[HLOC * g + h] / sqrt_d
        in_maps.append(
            {
                "xT": np.ascontiguousarray(x[b].T).astype(ml_dtypes.bfloat16),
                "wq": np.ascontiguousarray(Wq[:, cs]).astype(ml_dtypes.bfloat16),
                "wk": np.ascontiguousarray(Wk[:, cs]).astype(ml_dtypes.bfloat16),
                "wv": np.ascontiguousarray(Wv[:, cs]).astype(ml_dtypes.bfloat16),
                "wproj": np.ascontiguousarray(Wproj[cs, :]),
                "ucolblk": ucolblk,
                "omg": np.ascontiguousarray(omega_eff[hs][:, None]),
                "negomg": np.ascontiguousarray(-omega_eff[hs][:, None]),
                "iota4": np.tile(iota, (HLOC, 1)),
                "qrows": qrows,
                "ones64c": ones64c,
                "stairT": stairT,
                "ident": ident,
                "scat": scat,
                "selq": selq,
                "ones64": ones64,
                "ssqw4": ssqw4,
                "qw8": qw8,
                "kw": kw,
            }
        )
    return in_maps


def kernel(**inputs):
    from concourse.bass_utils import run_bass_kernel_spmd

    if "nc" not in _cache:
        _cache["nc"] = _build_program()
    nc = _cache["nc"]

    in_maps = _host_shards(inputs)
    res = run_bass_kernel_spmd(nc, in_maps, core_ids=list(range(8)))
    out = np.zeros((B, T, C), np.float32)
    for core in range(8):
        b = core // HLOC
        out[b] += res.results[core]["out"]
    return out


# revision 31
# speedup vs baseline: 1.8438x; 1.0208x over previous
"""Causal self-attention (RMSNorm QK, key-gated ALiBi bias) on 8 TRN2 cores.

Sharding: data-parallel over batch (2) x tensor-parallel over heads (4 groups
of 4 heads) = 8 cores. Each core computes a partial c_proj output for its
batch; the host sums the 4 head-group partials per batch.

Device kernel v2 (restructured from the 347us baseline):
  - Inputs stream in as a few large rearranged DMAs; QKV matmuls start on
    the first T-chunk while later chunks load (kills the DMA-only lead-in).
  - RMS rsqrt batched: sum-of-squares rows for all 4 (pack, q/k) combos are
    stacked into one [8,512] PSUM tile by matmul, one Ln + one Exp per chunk.
  - rsqrt/denominator broadcasts over 64 partitions via PE selector matmuls
    (no DRAM roundtrips).
  - Key-gate softplus batched: gate logits for 4 heads stacked by matmul
    accumulation, one Exp + one Ln per chunk.
  - Bias rows (a_hi/a_lo/w_hi/w_lo) stacked head-major by matmul, one DVE
    copy + 4 DMAs per chunk.
  - Softmax denominator reciprocal on DVE (vector.reciprocal).
  - Causal stair mask folded into the score matmul accumulation group as a
    constant bf16 matmul (stairT^T @ I).
"""

import sys

if "/opt/trn_rl_repo" not in sys.path:
    sys.path.insert(0, "/opt/trn_rl_repo")

import math

import numpy as np

B, T, C = 2, 2048, 1024
H, D = 16, 64
HLOC = 4           # heads per core
HD = HLOC * D      # 256
NCH = 512          # T-chunk width
NT = T // NCH      # 4 chunks
JT = T // 128      # 16 j-tiles
KC = C // 128      # 8 contraction chunks
EPS_RMS = 1e-5
U_L2_EPS = 1e-6
NEG_BIG = -1.0e30

_cache = {}

# P4 emission config (sweepable)
CFG = {
    "paired": True,
    "alternate": True,
    "sps_bufs": 2,
    "p_bufs": 6,
    "rep_own": False,
    "pv_defer": 6,
}


def _get_alibi_slopes(n_heads):
    def pow2(n):
        start = 2 ** (-(2 ** (-(math.log2(n) - 3))))
        return [start * start**i for i in range(n)]

    if math.log2(n_heads).is_integer():
        return pow2(n_heads)
    c = 2 ** math.floor(math.log2(n_heads))
    s = pow2(c)
    extra = _get_alibi_slopes(2 * c)
    return s + extra[0::2][: n_heads - c]


def _build_program(cfg=None):
    cfg = dict(CFG if cfg is None else cfg)
    import concourse.bass as bass
    import concourse.mybir as mybir
    import concourse.tile as tile
    from concourse.alu_op_type import AluOpType
    from concourse.vector_clock import ScopedClock

    F32 = mybir.dt.float32
    F32R = mybir.dt.float32r
    BF16 = mybir.dt.bfloat16
    AF = mybir.ActivationFunctionType
    MUL = AluOpType.mult
    SUB = AluOpType.subtract

    class PatchedTileContext(tile.TileContext):
        """Tail drain split into nops carrying <=2 sem waits each (this
        walrus build rejects CTRL instructions with more)."""

        def _drain_and_barrier(self, tick_clock, wait_clock):
            nc = self.nc
            probe = nc.sync.nop(nofuse=True)
            wait_clock.add_sem_waits(
                probe.ins, ScopedClock({None: tick_clock.global_clock})
            )
            si = probe.ins.sync_info
            waits = list(si.on_wait or []) if si is not None else []
            if len(waits) > 2:
                si.on_wait = waits[:2]
                rest = waits[2:]
                for i in range(0, len(rest), 2):
                    extra = nc.sync.nop(nofuse=True)
                    esi = extra.ins.sync_info
                    chunk = rest[i : i + 2]
                    if esi is None:
                        extra.ins.sync_info = mybir.SyncInfo(
                            on_wait=chunk, on_update=[]
                        )
                    else:
                        esi.on_wait = (esi.on_wait or []) + chunk
            nc.sync.drain()
            nc.all_engine_barrier()
            assert self.sems is not None
            popped = nc._tile_sem_poison_stack.pop()
            assert popped is self._sem_poison
            nc.clear_and_free_semaphores(list(self.sems.allocated().values()))
            nc.all_engine_barrier()

    def split_excess_waits(nc, max_waits=1):
        for f in nc.m.functions:
            for blk in f.blocks:
                new_insts = []
                for inst in blk.instructions:
                    si = inst.sync_info
                    if si is not None and si.on_wait and len(si.on_wait) > max_waits:
                        waits = list(si.on_wait)
                        si.on_wait = waits[-max_waits:]
                        rest = waits[:-max_waits]
                        for i in range(0, len(rest), max_waits):
                            nop = mybir.InstNoOp(
                                name=f"I-waitsplit-{nc.next_id()}",
                                ins=[],
                                outs=[],
                                engine=inst.engine,
                                sync_info=mybir.SyncInfo(
                                    on_wait=rest[i : i + max_waits], on_update=[]
                                ),
                            )
                            nc.register_instruction(nop)
                            new_insts.append(nop)
                    new_insts.append(inst)
                blk.instructions = new_insts

    nc = bass.Bass(trn_type="TRN2", num_devices=8, debug=False)

    # ---- DRAM I/O (per-core shards supplied by the host) ----
    d_xT = nc.dram_tensor("xT", [C, T], BF16, kind="ExternalInput")
    d_wq = nc.dram_tensor("wq", [C, HD], BF16, kind="ExternalInput")
    d_wk = nc.dram_tensor("wk", [C, HD], BF16, kind="ExternalInput")
    d_wv = nc.dram_tensor("wv", [C, HD], BF16, kind="ExternalInput")
    d_wproj = nc.dram_tensor("wproj", [HD, C], F32, kind="ExternalInput")
    d_ucol8 = nc.dram_tensor("ucol8", [D, 128], F32, kind="ExternalInput")
    d_omg = nc.dram_tensor("omg", [8, 1], F32, kind="ExternalInput")
    d_negomg = nc.dram_tensor("negomg", [8, 1], F32, kind="ExternalInput")
    d_iota16 = nc.dram_tensor("iota16", [16, NCH], F32, kind="ExternalInput")
    d_qrows = nc.dram_tensor("qrows", [4, T], F32, kind="ExternalInput")
    d_ones64c = nc.dram_tensor("ones64c", [128, JT * HLOC], BF16, kind="ExternalInput")
    d_stairT = nc.dram_tensor("stairT", [128, 128], BF16, kind="ExternalInput")
    d_ident = nc.dram_tensor("ident", [128, 128], BF16, kind="ExternalInput")
    d_scat = nc.dram_tensor("scat", [8, 128], F32, kind="ExternalInput")
    d_selq = nc.dram_tensor("selq", [4, 256], F32, kind="ExternalInput")
    d_ones64 = nc.dram_tensor("ones64", [1, 64], F32, kind="ExternalInput")
    d_ssqw4 = nc.dram_tensor("ssqw4", [128, 8], F32, kind="ExternalInput")
    d_qw8 = nc.dram_tensor("qw8", [128, 1], F32, kind="ExternalInput")
    d_kw = nc.dram_tensor("kw", [128, 1], F32, kind="ExternalInput")
    d_out = nc.dram_tensor("out", [T, C], F32, kind="ExternalOutput")

    with PatchedTileContext(nc) as tc:
        from contextlib import ExitStack

        with ExitStack() as top:
            persist = top.enter_context(tc.tile_pool(name="persist", bufs=1))

            # ---- persistent SBUF tensors ----
            q_aug = [persist.tile([68, T], F32R, tag=f"qaug{h}", name=f"qaug{h}") for h in range(HLOC)]
            k_aug = [persist.tile([68, T], F32R, tag=f"kaug{h}", name=f"kaug{h}") for h in range(HLOC)]
            vbig = persist.tile([128, JT * HLOC * 65], BF16, tag="vbig", name="vbig")
            v_sb = [vbig[:, 260 * t : 260 * t + 260] for t in range(JT)]
            y_pack = [
                persist.tile([128, T], F32R, tag=f"ypk{p}", name=f"ypk{p}")
                for p in range(2)
            ]

            # ---- weights: one rearranged DMA each ----
            wq_sb = persist.tile([128, 2048], BF16, tag="wq", name="wq")
            wk_sb = persist.tile([128, 2048], BF16, tag="wk", name="wk")
            wv_sb = persist.tile([128, 2048], BF16, tag="wv", name="wv")
            wproj_sb = persist.tile([128, 2048], F32R, tag="wproj", name="wproj")
            def load_w(wsb, dten, half=None):
                halves = range(2) if half is None else [half]
                for hf in halves:
                    nc.sync.dma_start(
                        wsb[:, 1024 * hf : 1024 * hf + 1024].rearrange(
                            "p (c j) -> p c j", c=KC // 2
                        ),
                        dten[512 * hf : 512 * hf + 512, :].rearrange(
                            "(c p) j -> p c j", p=128
                        ),
                    )

            # ---- x chunks: 2 DMAs per T-chunk (4 contraction chunks each) ----
            xpool = top.enter_context(tc.tile_pool(name="xT", bufs=1))

            def load_xn_half(xt, n, hf):
                sl = slice(NCH * n, NCH * n + NCH)
                src = d_xT[512 * hf : 512 * hf + 512, sl].rearrange(
                    "(c p) t -> p c t", p=128
                )
                dst = xt[:, 2048 * hf : 2048 * hf + 2048].rearrange(
                    "p (c t) -> p c t", c=4
                )
                nc.sync.dma_start(dst, src)

            def load_xn(n):
                xt = xpool.tile([128, 4096], BF16, tag=f"x{n % 2}", name=f"x{n}")
                load_xn_half(xt, n, 0)
                load_xn_half(xt, n, 1)
                return xt

            # interleave the first x chunk with the q/k weights so the first
            # projection matmuls can start as early as possible; x1 right
            # after wk so chunk n=1 is never starved behind const DMAs
            load_w(wq_sb, d_wq, half=0)
            x0 = xpool.tile([128, 4096], BF16, tag="x0", name="x_0")
            load_xn_half(x0, 0, 0)
            load_w(wq_sb, d_wq, half=1)
            load_xn_half(x0, 0, 1)
            x_tiles = {0: x0}
            load_w(wk_sb, d_wk)
            x_tiles[1] = load_xn(1)
            load_w(wv_sb, d_wv)
            nc.sync.dma_start(
                wproj_sb[:].rearrange("p (g j) -> p g j", g=2),
                d_wproj[:].bitcast(F32R).rearrange("(g p) j -> p g j", p=128),
            )

            stairT = persist.tile([128, 128], BF16, tag="stairT", name="stairT")
            nc.sync.dma_start(stairT[:], d_stairT[:])
            ident = persist.tile([128, 128], BF16, tag="ident", name="ident")
            nc.sync.dma_start(ident[:], d_ident[:])
            scat = persist.tile([8, 128], F32R, tag="scat", name="scat")
            nc.sync.dma_start(scat[:], d_scat[:].bitcast(F32R))
            selq = persist.tile([4, 256], F32R, tag="selq", name="selq")
            nc.sync.dma_start(selq[:], d_selq[:].bitcast(F32R))
            ones64 = persist.tile([1, 64], F32R, tag="ones64", name="ones64")
            nc.sync.dma_start(ones64[:], d_ones64[:].bitcast(F32R))
            ssqw4 = persist.tile([128, 8], F32R, tag="ssqw4", name="ssqw4")
            nc.sync.dma_start(ssqw4[:], d_ssqw4[:].bitcast(F32R))
            ucol8 = persist.tile([D, 128], F32R, tag="ucol8", name="ucol8")
            nc.sync.dma_start(ucol8[:], d_ucol8[:].bitcast(F32R))
            omg = persist.tile([8, 1], F32, tag="omg", name="omg")
            nc.sync.dma_start(omg[:], d_omg[:])
            negomg = persist.tile([8, 1], F32, tag="negomg", name="negomg")
            nc.sync.dma_start(negomg[:], d_negomg[:])
            qw8 = persist.tile([128, 1], F32, tag="qw8", name="qw8")
            nc.sync.dma_start(qw8[:], d_qw8[:])
            kw = persist.tile([128, 1], F32, tag="kw", name="kw")
            nc.sync.dma_start(kw[:], d_kw[:])

            # q_aug fixed rows 64:68 = ones, ones, iota, iota
            for h in range(HLOC):
                nc.sync.dma_start(q_aug[h][64:68, :], d_qrows[:].bitcast(F32R))
            # v ones columns: one strided DMA over the whole v tile
            nc.sync.dma_start(
                vbig[:].rearrange("p (th d) -> p th d", d=65)[:, :, 64:65],
                d_ones64c[:].rearrange("p (th o) -> p th o", o=1),
            )

            iota8 = [persist.tile([8, NCH], F32, tag=f"iota8{hf}", name=f"iota8{hf}") for hf in range(2)]
            for hf in range(2):
                nc.sync.dma_start(iota8[hf][:], d_iota16[8 * hf : 8 * hf + 8, :])
            eps8 = persist.tile([8, 1], F32, tag="eps8", name="eps8")
            nc.vector.memset(eps8[:], EPS_RMS)
            neghalf8 = persist.tile([8, 1], F32, tag="neghalf8", name="neghalf8")
            nc.vector.memset(neghalf8[:], -0.5)
            one8 = persist.tile([8, 1], F32, tag="one8", name="one8")
            nc.vector.memset(one8[:], 1.0)

            # ================= P2+P3: QKV, rms, gate, bias rows ============
            with ExitStack() as p2:
                qk_ps = p2.enter_context(
                    tc.tile_pool(name="qkps", bufs=3, space="PSUM")
                )
                rep_ps = p2.enter_context(
                    tc.tile_pool(name="repps", bufs=1, space="PSUM")
                )
                v_ps = p2.enter_context(
                    tc.tile_pool(name="vps", bufs=2, space="PSUM")
                )
                s8_ps = p2.enter_context(
                    tc.tile_pool(name="s8ps", bufs=1, space="PSUM")
                )
                st_ps = p2.enter_context(
                    tc.tile_pool(name="stps", bufs=1, space="PSUM")
                )
                sq_pool = p2.enter_context(tc.tile_pool(name="qsq", bufs=2))
                rep_sb = p2.enter_context(tc.tile_pool(name="repS", bufs=2))
                rsq_pool = p2.enter_context(tc.tile_pool(name="rsq", bufs=2))
                g_pool = p2.enter_context(tc.tile_pool(name="gate", bufs=1))
                st_pool = p2.enter_context(tc.tile_pool(name="stsb", bufs=1))

                def qk_group(p, loc, xt):
                    """Projection group for (pack p, q/k loc) + its square."""
                    wsb = wk_sb if loc else wq_sb
                    ps = qk_ps.tile([128, NCH], F32, tag="qk", name="qk")
                    for cc in range(KC):
                        nc.tensor.matmul(
                            ps[:],
                            wsb[:, 256 * cc + 128 * p : 256 * cc + 128 * p + 128],
                            xt[:, 512 * cc : 512 * cc + 512],
                            start=(cc == 0),
                            stop=(cc == KC - 1),
                        )
                    qsq = sq_pool.tile([128, NCH], F32R, tag="qsq", name="qsq")
                    nc.scalar.activation(qsq[:], ps[:], AF.Square)
                    return ps, qsq

                def emit_rsq(n, p, qsq_list):
                    """Batched rsqrt of the two mean-squares."""
                    s4 = s8_ps.tile([4, NCH], F32, tag="s8", name=f"s4_{n}_{p}")
                    for loc, qsq in enumerate(qsq_list):
                        nc.tensor.matmul(
                            s4[:],
                            ssqw4[:, 4 * loc : 4 * loc + 4],
                            qsq[:],
                            start=(loc == 0),
                            stop=(loc == 1),
                            skip_group_check=True,
                        )
                    rsq_f = rsq_pool.tile([4, NCH], F32, tag="rsqf", name="rsqf")
                    nc.scalar.activation(rsq_f[:], s4[:], AF.Ln, bias=eps8[0:4, :])
                    rsq = rsq_pool.tile([4, NCH], F32R, tag="rsq", name="rsq")
                    nc.scalar.activation(
                        rsq[:], rsq_f[:], AF.Exp, scale=neghalf8[0:4, :]
                    )
                    return rsq

                def rms_apply(p, ps_list, rsq, sl):
                    for loc, ps in enumerate(ps_list):
                        rep = rep_ps.tile([128, NCH], F32, tag="rep", name="rep")
                        nc.tensor.matmul(
                            rep[:],
                            selq[:, 128 * loc : 128 * loc + 128],
                            rsq[:],
                            start=True,
                            stop=True,
                        )
                        repS = rep_sb.tile([128, NCH], F32, tag="repS", name="repS")
                        nc.scalar.copy(repS[:], rep[:])
                        wcol = kw if loc else qw8
                        aug_set = k_aug if loc else q_aug
                        for s in range(2):
                            nc.vector.scalar_tensor_tensor(
                                aug_set[2 * p + s][0:64, sl],
                                ps[64 * s : 64 * s + 64, :],
                                wcol[64 * s : 64 * s + 64, :],
                                repS[64 * s : 64 * s + 64, :],
                                MUL,
                                MUL,
                            )

                def v_group(n, tl, xt):
                    t = 4 * n + tl
                    vps = v_ps.tile([128, NCH], F32, tag="vq", name="vps")
                    for cc in range(KC):
                        nc.tensor.matmul(
                            vps[:, 0:HD],
                            xt[:, 512 * cc + 128 * tl : 512 * cc + 128 * tl + 128],
                            wv_sb[:, 256 * cc : 256 * cc + 256],
                            start=(cc == 0),
                            stop=(cc == KC - 1),
                        )
                    dst = v_sb[t].rearrange("p (h d) -> p h d", h=HLOC)[:, :, 0:64]
                    nc.scalar.copy(
                        dst, vps[:, 0:HD].rearrange("p (h d) -> p h d", h=HLOC)
                    )

                def emit_p3_half(hf):
                    """Gate + bias rows for chunks (2*hf, 2*hf+1), stacked
                    [8,512] so the ACT/DVE chain runs once per half."""
                    g8 = s8_ps.tile([8, NCH], F32, tag="s8", name=f"g8_{hf}")
                    cnt = 0
                    for j in range(2):
                        n = 2 * hf + j
                        sl = slice(NCH * n, NCH * n + NCH)
                        for h in range(HLOC):
                            nc.tensor.matmul(
                                g8[:],
                                ucol8[:, 8 * (4 * j + h) : 8 * (4 * j + h) + 8],
                                k_aug[h][0:64, sl],
                                start=(cnt == 0),
                                stop=(cnt == 7),
                                skip_group_check=True,
                            )
                            cnt += 1
                    gsc = g_pool.tile([8, NCH], F32, tag="gsc", name="gsc")
                    nc.scalar.activation(gsc[:], g8[:], AF.Exp)
                    gate8 = g_pool.tile([8, NCH], F32, tag="gate8", name="gate8")
                    nc.scalar.activation(gate8[:], gsc[:], AF.Ln, bias=one8[:])
                    a4f = g_pool.tile([8, NCH], F32, tag="a4f", name="a4f")
                    nc.vector.scalar_tensor_tensor(
                        a4f[:], gate8[:], omg[:], iota8[hf][:], MUL, MUL
                    )
                    w4f = g_pool.tile([8, NCH], F32, tag="w4f", name="w4f")
                    nc.vector.tensor_scalar_mul(w4f[:], gate8[:], negomg[:])
                    a_hi = g_pool.tile([8, NCH], F32R, tag="a_hi", name="a_hi")
                    nc.vector.tensor_copy(a_hi[:], a4f[:])
                    w_hi = g_pool.tile([8, NCH], F32R, tag="w_hi", name="w_hi")
                    nc.vector.tensor_copy(w_hi[:], w4f[:])
                    a_lo = g_pool.tile([8, NCH], F32R, tag="a_lo", name="a_lo")
                    nc.vector.scalar_tensor_tensor(
                        a_lo[:], a4f[:], 1.0, a_hi[:].bitcast(F32), MUL, SUB
                    )
                    w_lo = g_pool.tile([8, NCH], F32R, tag="w_lo", name="w_lo")
                    nc.vector.scalar_tensor_tensor(
                        w_lo[:], w4f[:], 1.0, w_hi[:].bitcast(F32), MUL, SUB
                    )
                    # stack: st32 rows 16j + 4h + r = S_r[4j+h]
                    st32 = st_ps.tile([32, NCH], F32, tag="st16", name="st32")
                    for r, srcr in enumerate((a_hi, a_lo, w_hi, w_lo)):
                        nc.tensor.matmul(
                            st32[:],
                            scat[:, 32 * r : 32 * r + 32],
                            srcr[:],
                            start=(r == 0),
                            stop=(r == 3),
                            skip_group_check=True,
                        )
                    stsb = st_pool.tile([32, NCH], F32R, tag="stsb", name="stsb")
                    nc.vector.tensor_copy(stsb[:], st32[:])
                    for j in range(2):
                        n = 2 * hf + j
                        sl = slice(NCH * n, NCH * n + NCH)
                        for h in range(HLOC):
                            nc.sync.dma_start(
                                k_aug[h][64:68, sl],
                                stsb[16 * j + 4 * h : 16 * j + 4 * h + 4, :],
                            )

                for n in range(NT):
                    sl = slice(NCH * n, NCH * n + NCH)
                    if n + 2 < NT:
                        x_tiles[n + 2] = load_xn(n + 2)
                    xt = x_tiles.pop(n)

                    # just-in-time emission: dependent matmuls placed late so
                    # they never saturate the PE wait queue; previous chunk's
                    # P3 chain overlaps this chunk's projections
                    psA0, qA0 = qk_group(0, 0, xt)
                    psA1, qA1 = qk_group(0, 1, xt)
                    v_group(n, 0, xt)
                    rsqA = emit_rsq(n, 0, [qA0, qA1])
                    v_group(n, 1, xt)
                    psB0, qB0 = qk_group(1, 0, xt)
                    rms_apply(0, [psA0, psA1], rsqA, sl)
                    psB1, qB1 = qk_group(1, 1, xt)
                    v_group(n, 2, xt)
                    rsqB = emit_rsq(n, 1, [qB0, qB1])
                    v_group(n, 3, xt)
                    rms_apply(1, [psB0, psB1], rsqB, sl)
                    if n == 2:
                        emit_p3_half(0)
                emit_p3_half(1)

            # ================= P4 + P5: attention & projection =============
            # Score tiles processed in PAIRS living in [128,1024] 2-bank PSUM
            # tiles; one Exp per pair. Diagonal pairs exp a few extra
            # (never-read) columns so the access pattern stays rectangular.
            with ExitStack() as p4:
                s_ps_pool = p4.enter_context(
                    tc.tile_pool(name="sps2", bufs=cfg["sps_bufs"], space="PSUM")
                )
                y_ps_pool = p4.enter_context(
                    tc.tile_pool(name="yps", bufs=2, space="PSUM")
                )
                o_ps_pool = p4.enter_context(
                    tc.tile_pool(name="ops", bufs=2, space="PSUM")
                )
                if cfg.get("rep_own"):
                    r_ps_pool = p4.enter_context(
                        tc.tile_pool(name="rps", bufs=1, space="PSUM")
                    )
                else:
                    r_ps_pool = o_ps_pool
                p_pool = p4.enter_context(tc.tile_pool(name="p", bufs=cfg["p_bufs"]))
                rcp_pool = p4.enter_context(tc.tile_pool(name="rcp", bufs=2))
                rep4_sb = p4.enter_context(tc.tile_pool(name="rep4", bufs=2))
                out_pool = p4.enter_context(tc.tile_pool(name="osb", bufs=2))

                def score_tile_mms(ci, h, dst, base, tj, smt_cap=None):
                    r = tj - 4 * ci
                    off = 0 if r < 0 else 128 * r
                    smt = min(off, 256)
                    if smt_cap is not None:
                        smt = min(smt, smt_cap)
                    nc.tensor.matmul(
                        dst[:, base + smt : base + NCH],
                        k_aug[h][:, 128 * tj : 128 * tj + 128],
                        q_aug[h][:, NCH * ci + smt : NCH * ci + NCH],
                        start=True,
                        stop=(r < 0),
                        skip_group_check=True,
                    )
                    if r >= 0:
                        # causal stair mask via constant matmul
                        nc.tensor.matmul(
                            dst[:, base + off : base + off + 128],
                            stairT[:],
                            ident[:],
                            start=False,
                            stop=True,
                            skip_group_check=True,
                        )
                    return off

                def pv_mm(ci, h, yps, psb, base, off, tj):
                    nc.tensor.matmul(
                        yps[:, off:NCH],
                        v_sb[tj][:, 65 * h : 65 * h + 65],
                        psb[:, base + off : base + NCH],
                        start=(tj == 0),
                        stop=(tj == 4 * ci + 3),
                        skip_group_check=True,
                    )

                def emit_score_pair(ci, h, yps, tj0, pending):
                    """Emit score mms + exp; PV matmuls are deferred by one
                    stage (pending list) so parked PVs never stall PE issue."""
                    if cfg["paired"]:
                        sps2 = s_ps_pool.tile(
                            [128, 2 * NCH], F32, tag="sps2", name="sps2"
                        )
                        r0 = tj0 - 4 * ci
                        cap = min(0 if r0 < 0 else 128 * r0, 256)
                        offs = [
                            score_tile_mms(ci, h, sps2, NCH * ti, tj0 + ti, cap)
                            for ti in range(2)
                        ]
                        # one exp for the pair, rectangular over both halves
                        # from min(offs) (extra cols never read)
                        eoff = offs[0]
                        psb = p_pool.tile(
                            [128, 2 * NCH], BF16, tag="p", name="p"
                        )
                        nc.scalar.activation(
                            psb[:]
                            .rearrange("p (t c) -> p t c", t=2)[:, :, eoff:NCH],
                            sps2[:]
                            .rearrange("p (t c) -> p t c", t=2)[:, :, eoff:NCH],
                            AF.Exp,
                        )
                        for ti in range(2):
                            pending.append(
                                (ci, h, yps, psb, NCH * ti, offs[ti], tj0 + ti)
                            )
                    else:
                        for ti in range(2):
                            tj = tj0 + ti
                            sps = s_ps_pool.tile(
                                [128, NCH], F32, tag="sps2", name="sps"
                            )
                            off = score_tile_mms(ci, h, sps, 0, tj)
                            psb = p_pool.tile(
                                [128, NCH], BF16, tag="p", name="p"
                            )
                            nc.scalar.activation(
                                psb[:, off:NCH], sps[:, off:NCH], AF.Exp
                            )
                            pending.append((ci, h, yps, psb, 0, off, tj))

                def flush_pv(pending, keep=0):
                    while len(pending) > keep:
                        pv_mm(*pending.pop(0))

                def emit_norm(ci, h, yps):
                    isl = slice(NCH * ci, NCH * ci + NCH)
                    p_pk, s_slot = divmod(h, 2)
                    # normalize: DVE reciprocal + PE broadcast (into the
                    # ops slot, which is idle until this chunk's P5)
                    rcp = rcp_pool.tile([1, NCH], F32R, tag="rcp", name="rcp")
                    with nc.allow_low_precision(reason="softmax denom bcast"):
                        nc.vector.reciprocal(rcp[:], yps[64:65, :])
                    repp = r_ps_pool.tile(
                        [128, NCH],
                        F32,
                        tag="rps" if cfg.get("rep_own") else "ops",
                        name="rep64",
                    )
                    nc.tensor.matmul(
                        repp[0:64, 0:NCH], ones64[:], rcp[:], start=True, stop=True
                    )
                    repS = rep4_sb.tile([64, NCH], F32, tag="rep4", name="rep4")
                    nc.vector.tensor_copy(repS[:], repp[0:64, 0:NCH])
                    nc.vector.tensor_tensor(
                        y_pack[p_pk][64 * s_slot : 64 * s_slot + 64, isl],
                        yps[0:64, :],
                        repS[:],
                        MUL,
                    )

                for ci in range(NT):
                    keep = cfg.get("pv_defer", 2)
                    if cfg["alternate"]:
                        # two heads in flight: alternate pair emission so one
                        # head's PE work hides the other's exp latency
                        for hp in range(2):
                            ha, hb = 2 * hp, 2 * hp + 1
                            ypsa = y_ps_pool.tile(
                                [65, NCH], F32, tag="yps", name="ypsa"
                            )
                            ypsb = y_ps_pool.tile(
                                [65, NCH], F32, tag="yps", name="ypsb"
                            )
                            pending = []
                            for tj0 in range(0, 4 * ci + 4, 2):
                                emit_score_pair(ci, ha, ypsa, tj0, pending)
                                flush_pv(pending, keep)
                                emit_score_pair(ci, hb, ypsb, tj0, pending)
                                flush_pv(pending, keep)
                            flush_pv(pending)
                            emit_norm(ci, ha, ypsa)
                            emit_norm(ci, hb, ypsb)
                    else:
                        for h in range(HLOC):
                            yps = y_ps_pool.tile(
                                [65, NCH], F32, tag="yps", name="yps"
                            )
                            pending = []
                            for tj0 in range(0, 4 * ci + 4, 2):
                                emit_score_pair(ci, h, yps, tj0, pending)
                                flush_pv(pending, keep)
                            flush_pv(pending)
                            emit_norm(ci, h, yps)
                    # P5: projection for the 4 t-tiles covered by this chunk
                    for tt in range(4 * ci, 4 * ci + 4):
                        osb = out_pool.tile([128, 1024], F32, tag="osb", name="osb")
                        for cn in range(2):
                            osl = slice(512 * cn, 512 * cn + 512)
                            ops = o_ps_pool.tile(
                                [128, NCH], F32, tag="ops", name="ops"
                            )
                            for p in range(2):
                                nc.tensor.matmul(
                                    ops[:],
                                    y_pack[p][:, 128 * tt : 128 * tt + 128],
                                    wproj_sb[:, 1024 * p + 512 * cn : 1024 * p + 512 * cn + 512],
                                    start=(p == 0),
                                    stop=(p == 1),
                                )
                            nc.vector.tensor_copy(osb[:, osl], ops[:])
                        nc.sync.dma_start(
                            d_out[128 * tt : 128 * tt + 128, :], osb[:]
                        )

    split_excess_waits(nc, max_waits=1)
    return nc


def _host_shards(inputs):
    x = np.asarray(inputs["x"], np.float32)
    Wq = np.asarray(inputs["Wq"], np.float32)
    Wk = np.asarray(inputs["Wk"], np.float32)
    Wv = np.asarray(inputs["Wv"], np.float32)
    Wproj = np.asarray(inputs["Wproj"], np.float32)
    q_rms_w = np.asarray(inputs["q_rms_w"], np.float32)
    k_rms_w = np.asarray(inputs["k_rms_w"], np.float32)
    omega = np.asarray(inputs["omega"], np.float32)
    u = np.asarray(inputs["u"], np.float32)

    import ml_dtypes

    slopes = np.asarray(_get_alibi_slopes(H), np.float32)
    omega_eff = np.log1p(np.exp(omega)) * slopes  # softplus(omega) * slopes
    u_n = u / np.maximum(np.linalg.norm(u, axis=-1, keepdims=True), U_L2_EPS)
    sqrt_d = math.sqrt(D)

    iota = np.arange(T, dtype=np.float32)[None, :]
    qrows = np.concatenate(
        [np.ones((2, T), np.float32), np.tile(iota, (2, 1))], axis=0
    )
    ones64c = np.ones((128, JT * HLOC), np.float32).astype(ml_dtypes.bfloat16)
    ones64 = np.ones((1, 64), np.float32)
    # selq [4, 256]: block loc: selq[2*loc + (m>=64), 128*loc + m] = 1
    selq = np.zeros((4, 256), np.float32)
    for loc in range(2):
        for m in range(128):
            selq[2 * loc + (m >= 64), 128 * loc + m] = 1.0
    jj = np.arange(128, dtype=np.float32)
    stair = np.where(jj[None, :] >= jj[:, None], 0.0, NEG_BIG).astype(np.float32)
    stairT = stair.T.astype(ml_dtypes.bfloat16)
    ident = np.eye(128, dtype=np.float32).astype(ml_dtypes.bfloat16)
    # ssqw4 [128, 8]: block loc (cols 4*loc..+4): col 4*loc + 2*loc + s <- 1/D
    # on rows 64s.. (s4 rows are 2*loc + s)
    ssqw4 = np.zeros((128, 8), np.float32)
    for loc in range(2):
        for s in range(2):
            ssqw4[64 * s : 64 * s + 64, 4 * loc + 2 * loc + s] = 1.0 / D

    # scat [8, 128]: block r (32 cols): scat[4j+h, 32r + 16j + 4h + r] = 1
    scat = np.zeros((8, 128), np.float32)
    for r in range(4):
        for j in range(2):
            for h in range(4):
                scat[4 * j + h, 32 * r + 16 * j + 4 * h + r] = 1.0
    # iota16 [16, 512]: row 8*hf + 4*j + h, col c -> 512*(2*hf + j) + c
    iota16 = np.zeros((16, NCH), np.float32)
    for hf in range(2):
        for j in range(2):
            for h in range(4):
                iota16[8 * hf + 4 * j + h, :] = np.arange(NCH) + NCH * (2 * hf + j)
    qw8 = np.tile(q_rms_w / 8.0, 2)[:, None].astype(np.float32)
    kw = np.tile(k_rms_w, 2)[:, None].astype(np.float32)

    in_maps = []
    for core in range(8):
        b, g = divmod(core, HLOC)
        hs = slice(HLOC * g, HLOC * g + HLOC)
        cs = slice(HD * g, HD * g + HD)
        # ucol8 [64,128]: block blk=(4j+h) (8 cols): col 8*blk+4j+h = u_n[h]/sqrt(D)
        ucol8 = np.zeros((D, 128), np.float32)
        for j in range(2):
            for h in range(HLOC):
                blk = 4 * j + h
                ucol8[:, 8 * blk + 4 * j + h] = u_n[HLOC * g + h] / sqrt_d
        in_maps.append(
            {
                "xT": np.ascontiguousarray(x[b].T).astype(ml_dtypes.bfloat16),
                "wq": np.ascontiguousarray(Wq[:, cs]).astype(ml_dtypes.bfloat16),
                "wk": np.ascontiguousarray(Wk[:, cs]).astype(ml_dtypes.bfloat16),
                "wv": np.ascontiguousarray(Wv[:, cs]).astype(ml_dtypes.bfloat16),
                "wproj": np.ascontiguousarray(Wproj[cs, :]),
                "ucol8": ucol8,
                "omg": np.ascontiguousarray(np.tile(omega_eff[hs], 2)[:, None]),
                "negomg": np.ascontiguousarray(np.tile(-omega_eff[hs], 2)[:, None]),
                "iota16": iota16,
                "qrows": qrows,
                "ones64c": ones64c,
                "stairT": stairT,
                "ident": ident,
                "scat": scat,
                "selq": selq,
                "ones64": ones64,
                "ssqw4": ssqw4,
                "qw8": qw8,
                "kw": kw,
            }
        )
    return in_maps


def kernel(**inputs):
    from concourse.bass_utils import run_bass_kernel_spmd

    if "nc" not in _cache:
        _cache["nc"] = _build_program()
    nc = _cache["nc"]

    in_maps = _host_shards(inputs)
    res = run_bass_kernel_spmd(nc, in_maps, core_ids=list(range(8)))
    out = np.zeros((B, T, C), np.float32)
    for core in range(8):
        b = core // HLOC
        out[b] += res.results[core]["out"]
    return out
